# revision 1
# baseline (speedup 1.0000x reference)
"""Trainium2 Bass kernel for MllamaTextSdpaAttention (GQA + RoPE + causal SDPA).

Strategy: tensor-parallel over heads across 8 NeuronCores. Core c owns
q-heads [4c, 4c+4) and kv-head c (kv groups intact). Each core computes
hidden @ Wq/Wk/Wv slices, RoPE, causal attention for its heads, and its
row-slice of the Wo matmul, yielding a partial [T, DIM] output (bf16).
The host sums the 8 partials in f32.

Layout tricks:
- hidden_states is fed transposed ([DIM, T], bf16) so every projection
  matmul has the contraction dim (features) on partitions.
- Q/K projections produce Q^T/K^T directly (head_dim=128 on partitions).
- The RoPE even/odd pairing is de-interleaved by permuting Wq/Wk columns
  on the host, turning RoPE into a half-rotation: the partner element sits
  64 partitions away, reachable with plain partition-offset slices. The
  d-permutation cancels in q.k^T. The 1/sqrt(d) scale is folded into Q's
  cos/sin tables.
- Scores are computed TRANSPOSED: scT[k, q] = K_rot^T(tile).T @ Q_rot^T.
  exp(scT) is then directly the moving operand for the P@V matmul
  (out^T[d,q] = V[k,d].T @ expT[k,q]) -- no P transposes or PSUM->SBUF
  P copies. Softmax denominators come from a ones-vector matmul on the
  PE (sum over k = partition dim), and the 1/sum normalization is applied
  to the small out^T tile (via a PE-broadcast of the reciprocal row),
  not to P. No max-subtraction: scores are bounded (|s| <= ~20) so f32
  exp is safe, and masked entries use the additive -1e9 mask -> exp = 0.
- Causality at 128-block granularity: k-blocks strictly above the
  diagonal are never computed or read; diagonal blocks get the transposed
  additive mask from the actual attention_mask input.
- The 1/rowsum reciprocal row is broadcast across partitions on the idle
  GpSimd engine (partition_broadcast), and each group's normalization
  epilogue is deferred into the next group (software pipelining) so the
  PE never waits on the DVE reciprocal.
- Emission interleaves projection chunks with the attention groups they
  unblock (chunk0 -> b0/qb0 groups -> chunk1 -> b0/qb1 groups -> ...) and
  all [128,512]-f32 PSUM scratch (projection accumulators, score tiles,
  output accumulators) shares one 5-slot pool (+2 ot +1 rs = 8 banks)
  so the whole kernel fits PSUM without phase barriers.
- TimelineSim (instruction cost model): ~353 us/core; PE busy ~326 us
  (92% occupancy), which is the bf16 matmul-column floor for this
  decomposition.
"""

import numpy as np
import ml_dtypes

import concourse.bacc as bacc
import concourse.bass as bass
import concourse.mybir as mybir
from concourse.tile import TileContext
from concourse import bass_utils

BF16 = mybir.dt.bfloat16
F32 = mybir.dt.float32

B, S, DIM = 2, 1024, 4096
T = B * S                     # 2048 tokens, batch-major
N_HEADS, N_KV = 32, 8
HD = 128                      # head dim == partition count
N_CORES = 8
HL = N_HEADS // N_CORES       # 4 local q-heads per core
KT = DIM // 128               # 32 feature tiles
CH = 512                      # projection token-chunk
NCHUNK = T // CH
QB = 512                      # attention q-block width
TT = T // 128                 # 16 token tiles global
SCALE = 1.0 / float(np.sqrt(HD))

_CACHE: dict = {}


def _build():
    nc = bacc.Bacc("TRN2", target_bir_lowering=False, debug=False,
                   enable_asserts=False)

    hsT = nc.dram_tensor("hsT", [DIM, T], BF16, kind="ExternalInput")
    wq = nc.dram_tensor("wq", [DIM, HL * HD], BF16, kind="ExternalInput")
    wk = nc.dram_tensor("wk", [DIM, HD], BF16, kind="ExternalInput")
    wv = nc.dram_tensor("wv", [DIM, HD], BF16, kind="ExternalInput")
    wo = nc.dram_tensor("wo", [HL * HD, DIM], BF16, kind="ExternalInput")
    cos_q = nc.dram_tensor("cos_q", [HD, T], BF16, kind="ExternalInput")
    sin_q = nc.dram_tensor("sin_q", [HD, T], BF16, kind="ExternalInput")
    cos_k = nc.dram_tensor("cos_k", [HD, T], BF16, kind="ExternalInput")
    sin_k = nc.dram_tensor("sin_k", [HD, T], BF16, kind="ExternalInput")
    maskT = nc.dram_tensor("maskT", [128, 128], F32, kind="ExternalInput")
    out = nc.dram_tensor("out", [T, DIM], BF16, kind="ExternalOutput")

    Exp = mybir.ActivationFunctionType.Exp

    with TileContext(nc) as tc:
        with tc.tile_pool(name="consts", bufs=1) as cpool, \
             tc.tile_pool(name="hs", bufs=2) as hpool, \
             tc.tile_pool(name="rope_tmp", bufs=2) as rpool, \
             tc.tile_pool(name="work_ps", bufs=5, space=bass.MemorySpace.PSUM) as wpool, \
             tc.tile_pool(name="ot_ps", bufs=2, space=bass.MemorySpace.PSUM) as otpool, \
             tc.tile_pool(name="rs_ps", bufs=1, space=bass.MemorySpace.PSUM) as rspool, \
             tc.tile_pool(name="et", bufs=6) as epool, \
             tc.tile_pool(name="bc_sb", bufs=2) as bcsbpool, \
             tc.tile_pool(name="recip", bufs=4) as rcpool, \
             tc.tile_pool(name="out_sb", bufs=6) as xsbpool:

            wq_h = [cpool.tile([128, KT, HD], BF16, tag=f"wq{m}", name=f"wq{m}")
                    for m in range(HL)]
            wk_sb = cpool.tile([128, KT, HD], BF16, tag="wk")
            wv_sb = cpool.tile([128, KT, HD], BF16, tag="wv")
            cq_sb = cpool.tile([128, T], BF16, tag="cq")
            sq_sb = cpool.tile([128, T], BF16, tag="sq")
            ck_sb = cpool.tile([128, T], BF16, tag="ck")
            sk_sb = cpool.tile([128, T], BF16, tag="sk")
            maskT_sb = cpool.tile([128, 128], F32, tag="maskT")
            ones_k = cpool.tile([128, 1], BF16, tag="ones_k")
            qt_rot = cpool.tile([128, HL, T], BF16, tag="qt")
            kt_rot = cpool.tile([128, T], BF16, tag="kt")
            v_sb = cpool.tile([128, TT, HD], BF16, tag="v")
            ao = cpool.tile([128, HL, T], BF16, tag="ao")

            wq_r = wq.ap().rearrange("(kt p) n -> p kt n", p=128)
            hsT_r = hsT.ap().rearrange("(kt p) t -> p kt t", p=128)

            # startup-critical DMA first: the k-tiles the first matmuls touch
            nc.sync.dma_start(wq_h[0][:, 0:8, :], wq_r[:, 0:8, 0:HD])
            nc.sync.dma_start(wq_h[0][:, 8:KT, :], wq_r[:, 8:KT, 0:HD])

            def late_consts():
                nc.sync.dma_start(wq_h[1], wq_r[:, :, HD:2 * HD])
                nc.sync.dma_start(cq_sb, cos_q.ap())
                nc.sync.dma_start(sq_sb, sin_q.ap())
                for m in range(2, HL):
                    nc.sync.dma_start(wq_h[m], wq_r[:, :, m * HD:(m + 1) * HD])
                nc.sync.dma_start(wk_sb, wk.ap().rearrange("(kt p) n -> p kt n", p=128))
                nc.sync.dma_start(ck_sb, cos_k.ap())
                nc.sync.dma_start(sk_sb, sin_k.ap())
                nc.sync.dma_start(wv_sb, wv.ap().rearrange("(kt p) n -> p kt n", p=128))
                nc.sync.dma_start(maskT_sb, maskT.ap())
                nc.vector.memset(ones_k, 1.0)

            def rope(ps, out_ap, cos_ap, sin_ap):
                """out = ps*cos + halfswap(ps)*sin  (signs baked into sin)."""
                t1 = rpool.tile([128, CH], F32, tag="r1", name="t1")
                t2 = rpool.tile([128, CH], F32, tag="r2", name="t2")
                nc.vector.tensor_mul(t1, ps, cos_ap)
                nc.vector.tensor_mul(t2[0:64, :], ps[64:128, :], sin_ap[0:64, :])
                nc.vector.tensor_mul(t2[64:128, :], ps[0:64, :], sin_ap[64:128, :])
                nc.vector.tensor_add(out_ap, t1, t2)

            def emit_chunk(c):
                t0 = c * CH
                hs_sb = hpool.tile([128, KT, CH], BF16, tag="hs", name="hs_sb")
                for g in range(4):
                    nc.sync.dma_start(hs_sb[:, g * 8:(g + 1) * 8, :],
                                      hsT_r[:, g * 8:(g + 1) * 8, t0:t0 + CH])
                for m in range(HL):
                    ps = wpool.tile([128, CH], F32, tag="work", name="ps_q")
                    for kt in range(KT):
                        nc.tensor.matmul(ps, wq_h[m][:, kt, :], hs_sb[:, kt, :],
                                         start=(kt == 0), stop=(kt == KT - 1))
                    if c == 0 and m == 0:
                        late_consts()
                    rope(ps, qt_rot[:, m, t0:t0 + CH],
                         cq_sb[:, t0:t0 + CH], sq_sb[:, t0:t0 + CH])
                ps = wpool.tile([128, CH], F32, tag="work", name="ps_k")
                for kt in range(KT):
                    nc.tensor.matmul(ps, wk_sb[:, kt, :], hs_sb[:, kt, :],
                                     start=(kt == 0), stop=(kt == KT - 1))
                rope(ps, kt_rot[:, t0:t0 + CH],
                     ck_sb[:, t0:t0 + CH], sk_sb[:, t0:t0 + CH])
                for vi in range(CH // 128):
                    tt = t0 // 128 + vi
                    ps = wpool.tile([128, HD], F32, tag="work", name="ps_v")
                    for kt in range(KT):
                        nc.tensor.matmul(ps, hs_sb[:, kt, vi * 128:(vi + 1) * 128],
                                         wv_sb[:, kt, :],
                                         start=(kt == 0), stop=(kt == KT - 1))
                    nc.scalar.copy(v_sb[:, tt, :], ps)

            # --- attention group machinery (transposed-scores scheme) ---
            pending = [None]

            def epilogue(st):
                rs, ot, h, q0 = st
                recip = rcpool.tile([1, QB], F32, tag="recip", name="recip")
                nc.vector.reciprocal(recip, rs)
                bcs = bcsbpool.tile([128, QB], F32, tag="bcs", name="bcs")
                nc.gpsimd.partition_broadcast(bcs, recip)
                nc.vector.tensor_mul(ao[:, h, q0:q0 + QB], ot, bcs)

            def emit_group(b, h, qb):
                q0 = b * S + qb * QB
                n_kt = (qb + 1) * (QB // 128)
                rs = rspool.tile([1, QB], F32, tag="rs", name="rs")
                ot = otpool.tile([128, QB], F32, tag="ot", name="ot")
                ets = [None] * n_kt

                def emit_sc(kt):
                    c0 = max(0, kt - qb * (QB // 128)) * 128
                    sc = wpool.tile([128, QB], F32, tag="work", name="sc")
                    nc.tensor.matmul(
                        sc[:, c0:],
                        kt_rot[:, b * S + kt * 128:b * S + (kt + 1) * 128],
                        qt_rot[:, h, q0 + c0:q0 + QB],
                        start=True, stop=True)
                    jd = kt - qb * (QB // 128)
                    if 0 <= jd < QB // 128:
                        nc.vector.tensor_add(sc[:, jd * 128:(jd + 1) * 128],
                                             sc[:, jd * 128:(jd + 1) * 128],
                                             maskT_sb)
                    et = epool.tile([128, QB], BF16, tag="et", name="et")
                    nc.scalar.activation(et[:, c0:], sc[:, c0:], Exp,
                                         bias=0.0, scale=1.0)
                    ets[kt] = (et, c0)

                for w in range(min(4, n_kt)):
                    emit_sc(w)
                for kt in range(n_kt):
                    if kt + 4 < n_kt:
                        emit_sc(kt + 4)
                    et, c0 = ets[kt]
                    nc.tensor.matmul(rs[:, c0:], ones_k, et[:, c0:],
                                     start=(kt == 0), stop=(kt == n_kt - 1))
                    nc.tensor.matmul(ot[:, c0:], v_sb[:, b * (S // 128) + kt, :],
                                     et[:, c0:], start=(kt == 0),
                                     stop=(kt == n_kt - 1))
                    ets[kt] = None
                    if kt == 0 and pending[0] is not None:
                        epilogue(pending[0])
                        pending[0] = None
                pending[0] = (rs, ot, h, q0)

            # --- interleaved emission: each chunk unblocks a set of groups ---
            # chunk c covers tokens [c*512, (c+1)*512) = batch c//2, q-block c%2
            wo_sb = None
            for c in range(NCHUNK):
                emit_chunk(c)
                b, qb = c // 2, c % 2
                for h in range(HL):
                    emit_group(b, h, qb)
                if c == NCHUNK - 1:
                    # wo reuses an hs slot (same size); DMA overlaps the
                    # final attention groups
                    wo_sb = hpool.tile([128, HL, DIM], BF16, tag="hs",
                                       name="wo_sb")
                    nc.sync.dma_start(
                        wo_sb, wo.ap().rearrange("(kh p) n -> p kh n", p=128))
            if pending[0] is not None:
                epilogue(pending[0])
                pending[0] = None

            # ---- output projection (row-parallel Wo) ----
            for tt in range(TT):
                for ni, n0 in enumerate(range(0, DIM, 512)):
                    ps = wpool.tile([128, 512], F32, tag="work", name="ps_o")
                    for kh in range(HL):
                        nc.tensor.matmul(ps, ao[:, kh, tt * 128:(tt + 1) * 128],
                                         wo_sb[:, kh, n0:n0 + 512],
                                         start=(kh == 0), stop=(kh == HL - 1))
                    osb = xsbpool.tile([128, 512], BF16, tag="osb", name="osb")
                    if (tt * 8 + ni) % 2 == 0:
                        nc.scalar.copy(osb, ps)
                    else:
                        nc.vector.tensor_copy(osb, ps)
                    nc.sync.dma_start(out.ap()[tt * 128:(tt + 1) * 128,
                                               n0:n0 + 512], osb)
    nc.compile()
    return nc


def _get_nc():
    if "nc" not in _CACHE:
        _CACHE["nc"] = _build()
    return _CACHE["nc"]


def _prep_inputs(inputs) -> list[dict]:
    bf16 = ml_dtypes.bfloat16
    hs = np.asarray(inputs["hidden_states"], dtype=np.float32).reshape(T, DIM)
    hsT = np.ascontiguousarray(hs.T).astype(bf16)

    fc = np.asarray(inputs["freqs_cos"], dtype=np.float32).reshape(T, HD // 2).T
    fs = np.asarray(inputs["freqs_sin"], dtype=np.float32).reshape(T, HD // 2).T
    cos2 = np.concatenate([fc, fc], axis=0)            # [128, T]
    sin2 = np.concatenate([-fs, fs], axis=0)           # signed half-rotation
    cos_qv = np.ascontiguousarray(cos2 * SCALE).astype(bf16)
    sin_qv = np.ascontiguousarray(sin2 * SCALE).astype(bf16)
    cos_kv = np.ascontiguousarray(cos2).astype(bf16)
    sin_kv = np.ascontiguousarray(sin2).astype(bf16)

    maskT = np.ascontiguousarray(
        np.asarray(inputs["attention_mask"], dtype=np.float32)[0, 0, :128, :128].T)

    perm = np.concatenate([np.arange(0, HD, 2), np.arange(1, HD, 2)])
    Wq = np.asarray(inputs["Wq"], dtype=np.float32)
    Wk = np.asarray(inputs["Wk"], dtype=np.float32)
    Wv = np.asarray(inputs["Wv"], dtype=np.float32)
    Wo = np.asarray(inputs["Wo"], dtype=np.float32)

    in_maps = []
    for c in range(N_CORES):
        wq_c = np.concatenate(
            [Wq[:, (c * HL + h) * HD:(c * HL + h + 1) * HD][:, perm]
             for h in range(HL)], axis=1)
        wk_c = Wk[:, c * HD:(c + 1) * HD][:, perm]
        wv_c = Wv[:, c * HD:(c + 1) * HD]
        wo_c = Wo[c * HL * HD:(c + 1) * HL * HD, :]
        in_maps.append({
            "hsT": hsT,
            "wq": np.ascontiguousarray(wq_c).astype(bf16),
            "wk": np.ascontiguousarray(wk_c).astype(bf16),
            "wv": np.ascontiguousarray(wv_c).astype(bf16),
            "wo": np.ascontiguousarray(wo_c).astype(bf16),
            "cos_q": cos_qv, "sin_q": sin_qv,
            "cos_k": cos_kv, "sin_k": sin_kv,
            "maskT": maskT,
        })
    return in_maps


def kernel(**inputs) -> np.ndarray:
    nc = _get_nc()
    in_maps = _prep_inputs(inputs)
    res = bass_utils.run_bass_kernel_spmd(nc, in_maps,
                                          core_ids=list(range(N_CORES)))
    acc = np.zeros((T, DIM), dtype=np.float32)
    for c in range(N_CORES):
        acc += np.asarray(res.results[c]["out"], dtype=np.float32)
    return acc.reshape(B, S, DIM)



# revision 13
# speedup vs baseline: 1.2423x; 1.2423x over previous
"""Trainium2 Bass kernel for MllamaTextSdpaAttention (GQA + RoPE + causal SDPA).

Tensor-parallel over heads across 8 NeuronCores (core c owns q-heads
[4c, 4c+4) and kv-head c); each core emits a partial [T, DIM] output that the
host sums in f32.

Speed over the bf16 baseline comes from fp8e4 DoubleRow matmuls with hi/lo
error compensation on all four projections:
  x ~= x_hi + x_lo (both fp8e4);  A@B ~= Ah@Bh + Al@Bh + Ah@Bl
Weights store pairs as (lo, hi), activations as (hi, lo), so
  - main terms pair two k-tiles per DoubleRow instruction:
      lhsT = (Wh[2k], Wh[2k+1]), rhs = (Xh[2k], Xh[2k+1])
  - correction terms pair the two cross products of one k-tile:
      lhsT = (Wl[kt], Wh[kt]),  rhs = (Xh[kt], Xl[kt])
Per k-tile this is 0.75x the bf16 PE-column cost, with accuracy slightly
better than bf16 (validated on device). hidden_states and all weights are
split host-side; the attention output is split on-chip.

Attention core (transposed scores, bf16) matches the baseline, except the
softmax denominators: instead of a ones-row matmul streaming the full P
tiles (N=QB columns each), the row-sums are computed transposed
(lhsT = P-tile, rhs = ones column -> out [q,1]), which costs ~1 PE cycle per
instruction. A PE transpose converts the [128,4] column sums to rows for the
reciprocal + gpsimd partition-broadcast normalization path.

Scales: hidden*16 and weights*64 keep fp8 splits in the normal range; the
inverses are folded into the rope tables, the V copy, and the final output
copy.  ao is scaled by 8 via the ones vector (memset 1/8) for the same
reason.
"""

import numpy as np
import ml_dtypes

import concourse.bacc as bacc
import concourse.bass as bass
import concourse.mybir as mybir
from concourse.tile import TileContext
from concourse import bass_utils

BF16 = mybir.dt.bfloat16
F8 = mybir.dt.float8e4
F32 = mybir.dt.float32
DR = mybir.MatmulPerfMode.DoubleRow

B, S, DIM = 2, 1024, 4096
T = B * S                     # 2048 tokens, batch-major
N_HEADS, N_KV = 32, 8
HD = 128                      # head dim == partition count
N_CORES = 8
HL = N_HEADS // N_CORES       # 4 local q-heads per core
KT = DIM // 128               # 32 feature tiles
CH = 512                      # projection token-chunk
NCHUNK = T // CH
QB = 512                      # attention q-block width
TT = T // 128                 # 16 token tiles global
SCALE = 1.0 / float(np.sqrt(HD))
SH = 16.0                     # hidden prescale for fp8
SW = 64.0                     # weight prescale for fp8
SA = 8.0                      # ao prescale (via ones = 1/8)

_CACHE: dict = {}


def _build():
    nc = bacc.Bacc("TRN2", target_bir_lowering=False, debug=False,
                   enable_asserts=False)

    # fp8 pair layouts: weights (lo, hi) on the pair axis, activations
    # (hi, lo).  Free layout of each weight dram row matches its SBUF tile.
    hs8 = nc.dram_tensor("hs8", [DIM, 2, T], F8, kind="ExternalInput")
    wq8 = nc.dram_tensor("wq8", [HL * 128, KT * 2 * HD], F8, kind="ExternalInput")
    wk8 = nc.dram_tensor("wk8", [128, KT * 2 * HD], F8, kind="ExternalInput")
    wv8 = nc.dram_tensor("wv8", [128, KT * 2 * HD], F8, kind="ExternalInput")
    wo8 = nc.dram_tensor("wo8", [HL * 128, 2 * DIM], F8, kind="ExternalInput")
    cos_q = nc.dram_tensor("cos_q", [HD, T], BF16, kind="ExternalInput")
    sin_q = nc.dram_tensor("sin_q", [HD, T], BF16, kind="ExternalInput")
    cos_k = nc.dram_tensor("cos_k", [HD, T], BF16, kind="ExternalInput")
    sin_k = nc.dram_tensor("sin_k", [HD, T], BF16, kind="ExternalInput")
    maskT = nc.dram_tensor("maskT", [128, 128], F32, kind="ExternalInput")
    out = nc.dram_tensor("out", [T, DIM], BF16, kind="ExternalOutput")

    Exp = mybir.ActivationFunctionType.Exp

    with TileContext(nc) as tc:
        with tc.tile_pool(name="consts", bufs=1) as cpool, \
             tc.tile_pool(name="hs", bufs=2) as hpool, \
             tc.tile_pool(name="rope_tmp", bufs=2) as rpool, \
             tc.tile_pool(name="work_ps", bufs=5, space=bass.MemorySpace.PSUM) as wpool, \
             tc.tile_pool(name="ot_ps", bufs=2, space=bass.MemorySpace.PSUM) as otpool, \
             tc.tile_pool(name="rs_ps", bufs=1, space=bass.MemorySpace.PSUM) as rspool, \
             tc.tile_pool(name="et", bufs=6) as epool, \
             tc.tile_pool(name="rcp", bufs=2) as rcppool, \
             tc.tile_pool(name="rrow", bufs=2) as rrowpool, \
             tc.tile_pool(name="bc_sb", bufs=2) as bcsbpool, \
             tc.tile_pool(name="aobf", bufs=2) as aobfpool, \
             tc.tile_pool(name="out_sb", bufs=6) as xsbpool:

            wq_h = [cpool.tile([128, KT, 2, HD], F8, tag=f"wq{m}", name=f"wq{m}")
                    for m in range(HL)]
            wk_sb = cpool.tile([128, KT, 2, HD], F8, tag="wk")
            wv_sb = cpool.tile([128, KT, 2, HD], F8, tag="wv")
            cq_sb = cpool.tile([128, T], BF16, tag="cq")
            sq_sb = cpool.tile([128, T], BF16, tag="sq")
            ck_sb = cpool.tile([128, T], BF16, tag="ck")
            sk_sb = cpool.tile([128, T], BF16, tag="sk")
            maskT_sb = cpool.tile([128, 128], F32, tag="maskT")
            ones_k = cpool.tile([128, 1], BF16, tag="ones_k")
            qt_rot = cpool.tile([128, HL, T], BF16, tag="qt")
            kt_rot = cpool.tile([128, T], BF16, tag="kt")
            v_sb = cpool.tile([128, TT, HD], BF16, tag="v")
            ao8 = cpool.tile([128, HL, 2, T], F8, tag="ao")

            hs_r = hs8.ap().rearrange("(kt p) two t -> p kt two t", p=128)

            # startup-critical DMA first: the k-tiles the first matmuls touch
            nc.sync.dma_start(wq_h[0][:, 0:8, :, :],
                              wq8.ap()[0:128, 0:8 * 2 * HD]
                              .rearrange("p (kt two n) -> p kt two n", two=2, n=HD))
            nc.sync.dma_start(wq_h[0][:, 8:KT, :, :],
                              wq8.ap()[0:128, 8 * 2 * HD:KT * 2 * HD]
                              .rearrange("p (kt two n) -> p kt two n", two=2, n=HD))

            def late_consts():
                nc.sync.dma_start(
                    wq_h[1], wq8.ap()[128:256, :]
                    .rearrange("p (kt two n) -> p kt two n", two=2, n=HD))
                nc.sync.dma_start(cq_sb, cos_q.ap())
                nc.sync.dma_start(sq_sb, sin_q.ap())
                for m in range(2, HL):
                    nc.sync.dma_start(
                        wq_h[m], wq8.ap()[m * 128:(m + 1) * 128, :]
                        .rearrange("p (kt two n) -> p kt two n", two=2, n=HD))
                nc.sync.dma_start(
                    wk_sb, wk8.ap().rearrange("p (kt two n) -> p kt two n",
                                              two=2, n=HD))
                nc.sync.dma_start(ck_sb, cos_k.ap())
                nc.sync.dma_start(sk_sb, sin_k.ap())
                nc.sync.dma_start(
                    wv_sb, wv8.ap().rearrange("p (kt two n) -> p kt two n",
                                              two=2, n=HD))
                nc.sync.dma_start(maskT_sb, maskT.ap())
                nc.vector.memset(ones_k, 1.0 / SA)

            def rope(ps, out_ap, cos_ap, sin_ap):
                """out = ps*cos + halfswap(ps)*sin  (signs baked into sin)."""
                t1 = rpool.tile([128, CH], F32, tag="r1", name="t1")
                t2 = rpool.tile([128, CH], F32, tag="r2", name="t2")
                nc.vector.tensor_mul(t1, ps, cos_ap)
                nc.vector.tensor_mul(t2[0:64, :], ps[64:128, :], sin_ap[0:64, :])
                nc.vector.tensor_mul(t2[64:128, :], ps[0:64, :], sin_ap[64:128, :])
                nc.vector.tensor_add(out_ap, t1, t2)

            def proj_mms(ps, w_sb, hs_sb, n0, n1, vi=None):
                """Emit the fp8-compensated matmuls contracting k-tiles
                [n0, n1) of w against hs chunk columns (or token block vi for
                the V orientation where hs is stationary)."""
                mms = []
                for k2 in range(n0 // 2, n1 // 2):
                    if vi is None:
                        mms.append((w_sb[:, 2 * k2:2 * k2 + 2, 1, :],
                                    hs_sb[:, 2 * k2:2 * k2 + 2, 0, :]))
                    else:
                        blk = slice(vi * 128, (vi + 1) * 128)
                        mms.append((hs_sb[:, 2 * k2:2 * k2 + 2, 0, blk],
                                    w_sb[:, 2 * k2:2 * k2 + 2, 1, :]))
                for kt in range(n0, n1):
                    if vi is None:
                        mms.append((w_sb[:, kt, :, :], hs_sb[:, kt, :, :]))
                    else:
                        blk = slice(vi * 128, (vi + 1) * 128)
                        mms.append((hs_sb[:, kt, :, blk], w_sb[:, kt, :, :]))
                return mms

            def emit_chunk(c):
                t0 = c * CH
                hs_sb = hpool.tile([128, KT, 2, CH], F8, tag="hs", name="hs_sb")
                for g in range(4):
                    for s in range(2):
                        nc.sync.dma_start(
                            hs_sb[:, g * 8:(g + 1) * 8, s, :],
                            hs_r[:, g * 8:(g + 1) * 8, s, t0:t0 + CH])
                for m in range(HL):
                    ps = wpool.tile([128, CH], F32, tag="work", name="ps_q")
                    mms = proj_mms(ps, wq_h[m], hs_sb, 0, KT)
                    for i, (lhsT, rhs) in enumerate(mms):
                        nc.tensor.matmul(ps, lhsT, rhs, start=(i == 0),
                                         stop=(i == len(mms) - 1), perf_mode=DR)
                    if c == 0 and m == 0:
                        late_consts()
                    rope(ps, qt_rot[:, m, t0:t0 + CH],
                         cq_sb[:, t0:t0 + CH], sq_sb[:, t0:t0 + CH])
                ps = wpool.tile([128, CH], F32, tag="work", name="ps_k")
                mms = proj_mms(ps, wk_sb, hs_sb, 0, KT)
                for i, (lhsT, rhs) in enumerate(mms):
                    nc.tensor.matmul(ps, lhsT, rhs, start=(i == 0),
                                     stop=(i == len(mms) - 1), perf_mode=DR)
                rope(ps, kt_rot[:, t0:t0 + CH],
                     ck_sb[:, t0:t0 + CH], sk_sb[:, t0:t0 + CH])
                for vi in range(CH // 128):
                    tt = t0 // 128 + vi
                    ps = wpool.tile([128, HD], F32, tag="work", name="ps_v")
                    mms = proj_mms(ps, wv_sb, hs_sb, 0, KT, vi=vi)
                    for i, (lhsT, rhs) in enumerate(mms):
                        nc.tensor.matmul(ps, lhsT, rhs, start=(i == 0),
                                         stop=(i == len(mms) - 1), perf_mode=DR)
                    nc.scalar.mul(v_sb[:, tt, :], ps, 1.0 / (SH * SW))

            # --- attention group machinery (transposed-scores scheme) ---
            pending = [None]

            def epilogue_rest(st):
                """Normalize ot by the per-token reciprocal row-sums and split
                the result into fp8 (hi, lo) for the O projection."""
                ot, h, q0, rrow = st
                bcs = bcsbpool.tile([128, QB], F32, tag="bcs", name="bcs")
                for j in range(4):
                    nc.gpsimd.partition_broadcast(bcs[:, j * 128:(j + 1) * 128],
                                                  rrow[0:1, :, j])
                abf = aobfpool.tile([128, QB], BF16, tag="abf", name="abf")
                nc.vector.tensor_mul(abf, ot, bcs)
                nc.vector.tensor_copy(ao8[:, h, 0, q0:q0 + QB], abf)
                nc.vector.tensor_sub(ao8[:, h, 1, q0:q0 + QB], abf,
                                     ao8[:, h, 0, q0:q0 + QB])

            def emit_group(b, h, qb):
                q0 = b * S + qb * QB
                n_kt = (qb + 1) * (QB // 128)
                rs = rspool.tile([128, 4], F32, tag="rs", name="rs")
                ot = otpool.tile([128, QB], F32, tag="ot", name="ot")
                ets = [None] * n_kt

                def emit_sc(kt):
                    c0 = max(0, kt - qb * (QB // 128)) * 128
                    sc = wpool.tile([128, QB], F32, tag="work", name="sc")
                    nc.tensor.matmul(
                        sc[:, c0:],
                        kt_rot[:, b * S + kt * 128:b * S + (kt + 1) * 128],
                        qt_rot[:, h, q0 + c0:q0 + QB],
                        start=True, stop=True)
                    jd = kt - qb * (QB // 128)
                    if 0 <= jd < QB // 128:
                        nc.vector.tensor_add(sc[:, jd * 128:(jd + 1) * 128],
                                             sc[:, jd * 128:(jd + 1) * 128],
                                             maskT_sb)
                    et = epool.tile([128, QB], BF16, tag="et", name="et")
                    nc.scalar.activation(et[:, c0:], sc[:, c0:], Exp,
                                         bias=0.0, scale=1.0)
                    ets[kt] = (et, c0)

                for w in range(min(4, n_kt)):
                    emit_sc(w)
                for kt in range(n_kt):
                    if kt + 4 < n_kt:
                        emit_sc(kt + 4)
                    et, c0 = ets[kt]
                    # transposed row-sums: one near-free matmul per q-subblock.
                    # PSUM lazy-zeroing is bank-granular: exactly one
                    # start=True per group; later first-writes to other
                    # columns auto-zero via the pending-zero region.
                    for qs in range(c0 // 128, 4):
                        nc.tensor.matmul(rs[:, qs:qs + 1],
                                         et[:, qs * 128:(qs + 1) * 128], ones_k,
                                         start=(kt == 0 and qs == 0),
                                         stop=(kt == n_kt - 1 and qs == 3),
                                         skip_group_check=True)
                    nc.tensor.matmul(ot[:, c0:], v_sb[:, b * (S // 128) + kt, :],
                                     et[:, c0:], start=(kt == 0),
                                     stop=(kt == n_kt - 1))
                    ets[kt] = None
                    if kt == 2 and pending[0] is not None:
                        epilogue_rest(pending[0])
                        pending[0] = None
                # stage this group's epilogue: reciprocal of the [128, 4]
                # column sums, then a tiny DMA transposes them into a row
                # ([1, 128, 4]: token p of subblock j at free offset 4p+j);
                # the gpsimd broadcast + normalization run inside the next
                # group off that row.
                rcp = rcppool.tile([128, 4], F32, tag="rcp", name="rcp")
                nc.vector.reciprocal(rcp, rs)
                rrow = rrowpool.tile([1, 128, 4], F32, tag="rr", name="rrow")
                nc.sync.dma_start(rrow[0:1, :, :], rcp)
                pending[0] = (ot, h, q0, rrow)

            # --- interleaved emission: each chunk unblocks a set of groups ---
            wo_sb = None
            for c in range(NCHUNK):
                emit_chunk(c)
                b, qb = c // 2, c % 2
                for h in range(HL):
                    emit_group(b, h, qb)
                if c == NCHUNK - 1:
                    # wo reuses an hs slot (same size); DMA overlaps the
                    # final attention groups
                    wo_sb = hpool.tile([128, HL, 2, DIM], F8, tag="hs",
                                       name="wo_sb")
                    for m in range(HL):
                        nc.sync.dma_start(wo_sb[:, m, :, :],
                                          wo8.ap()[m * 128:(m + 1) * 128, :]
                                          .rearrange("p (two n) -> p two n",
                                                     two=2))
            if pending[0] is not None:
                epilogue_rest(pending[0])
                pending[0] = None

            # ---- output projection (row-parallel Wo, fp8 compensated) ----
            for tt in range(TT):
                tb = slice(tt * 128, (tt + 1) * 128)
                for ni, n0 in enumerate(range(0, DIM, 512)):
                    ps = wpool.tile([128, 512], F32, tag="work", name="ps_o")
                    mms = []
                    for j in range(HL // 2):
                        mms.append((ao8[:, 2 * j:2 * j + 2, 0, tb],
                                    wo_sb[:, 2 * j:2 * j + 2, 1, n0:n0 + 512]))
                    for kh in range(HL):
                        mms.append((ao8[:, kh, :, tb],
                                    wo_sb[:, kh, :, n0:n0 + 512]))
                    for i, (lhsT, rhs) in enumerate(mms):
                        nc.tensor.matmul(ps, lhsT, rhs, start=(i == 0),
                                         stop=(i == len(mms) - 1), perf_mode=DR)
                    osb = xsbpool.tile([128, 512], BF16, tag="osb", name="osb")
                    if (tt * 8 + ni) % 2 == 0:
                        nc.scalar.mul(osb, ps, 1.0 / (SW * SA))
                    else:
                        nc.vector.tensor_scalar_mul(osb, ps, 1.0 / (SW * SA))
                    nc.sync.dma_start(out.ap()[tb, n0:n0 + 512], osb)
    nc.compile()
    return nc


def _get_nc():
    if "nc" not in _CACHE:
        _CACHE["nc"] = _build()
    return _CACHE["nc"]


def _split8(x):
    f8 = ml_dtypes.float8_e4m3
    hi = x.astype(f8)
    lo = (x - hi.astype(np.float32)).astype(f8)
    return hi, lo


def _prep_inputs(inputs) -> list[dict]:
    bf16 = ml_dtypes.bfloat16
    hs = np.asarray(inputs["hidden_states"], dtype=np.float32).reshape(T, DIM)
    hsT = np.ascontiguousarray(hs.T) * SH
    h_hi, h_lo = _split8(hsT)
    hs8 = np.ascontiguousarray(np.stack([h_hi, h_lo], axis=1))  # [DIM, 2, T]

    fc = np.asarray(inputs["freqs_cos"], dtype=np.float32).reshape(T, HD // 2).T
    fs = np.asarray(inputs["freqs_sin"], dtype=np.float32).reshape(T, HD // 2).T
    cos2 = np.concatenate([fc, fc], axis=0)            # [128, T]
    sin2 = np.concatenate([-fs, fs], axis=0)           # signed half-rotation
    qs = SCALE / (SH * SW)
    ks = 1.0 / (SH * SW)
    cos_qv = np.ascontiguousarray(cos2 * qs).astype(bf16)
    sin_qv = np.ascontiguousarray(sin2 * qs).astype(bf16)
    cos_kv = np.ascontiguousarray(cos2 * ks).astype(bf16)
    sin_kv = np.ascontiguousarray(sin2 * ks).astype(bf16)

    maskT = np.ascontiguousarray(
        np.asarray(inputs["attention_mask"], dtype=np.float32)[0, 0, :128, :128].T)

    perm = np.concatenate([np.arange(0, HD, 2), np.arange(1, HD, 2)])
    Wq = np.asarray(inputs["Wq"], dtype=np.float32)
    Wk = np.asarray(inputs["Wk"], dtype=np.float32)
    Wv = np.asarray(inputs["Wv"], dtype=np.float32)
    Wo = np.asarray(inputs["Wo"], dtype=np.float32)

    def pack_w(w, nheads):
        # w: [DIM, nheads*128] prescaled; -> [nheads*128p, KT*2*HD] with
        # (lo, hi) pairs: arr[m*128+p, ((kt*2)+s)*128+hd]
        hi, lo = _split8(w)
        pair = np.stack([lo.astype(np.float32), hi.astype(np.float32)], axis=1)
        # [DIM, 2, nheads*128] -> [KT, 128p, 2, nheads, 128hd]
        v = pair.reshape(KT, 128, 2, nheads, HD)
        arr = v.transpose(3, 1, 0, 2, 4).reshape(nheads * 128, KT * 2 * HD)
        return np.ascontiguousarray(arr).astype(ml_dtypes.float8_e4m3)

    in_maps = []
    for c in range(N_CORES):
        wq_c = np.concatenate(
            [Wq[:, (c * HL + h) * HD:(c * HL + h + 1) * HD][:, perm]
             for h in range(HL)], axis=1) * SW
        wk_c = Wk[:, c * HD:(c + 1) * HD][:, perm] * SW
        wv_c = Wv[:, c * HD:(c + 1) * HD] * SW
        wo_c = Wo[c * HL * HD:(c + 1) * HL * HD, :] * SW

        o_hi, o_lo = _split8(wo_c)
        wo_pack = np.concatenate([o_lo.astype(np.float32),
                                  o_hi.astype(np.float32)],
                                 axis=1)  # [512, 2*DIM] (lo block, hi block)
        wo_pack = np.ascontiguousarray(wo_pack).astype(ml_dtypes.float8_e4m3)

        in_maps.append({
            "hs8": hs8,
            "wq8": pack_w(wq_c, HL),
            "wk8": pack_w(wk_c, 1),
            "wv8": pack_w(wv_c, 1),
            "wo8": wo_pack,
            "cos_q": cos_qv, "sin_q": sin_qv,
            "cos_k": cos_kv, "sin_k": sin_kv,
            "maskT": maskT,
        })
    return in_maps


def kernel(**inputs) -> np.ndarray:
    nc = _get_nc()
    in_maps = _prep_inputs(inputs)
    res = bass_utils.run_bass_kernel_spmd(nc, in_maps,
                                          core_ids=list(range(N_CORES)))
    acc = np.zeros((T, DIM), dtype=np.float32)
    for c in range(N_CORES):
        acc += np.asarray(res.results[c]["out"], dtype=np.float32)
    return acc.reshape(B, S, DIM)


# revision 25
# speedup vs baseline: 1.2695x; 1.0219x over previous
"""Trainium2 Bass kernel for MllamaTextSdpaAttention (GQA + RoPE + causal SDPA).

Tensor-parallel over heads across 8 NeuronCores (core c owns q-heads
[4c, 4c+4) and kv-head c); each core emits a partial [T, DIM] output that the
host sums in f32.

Speed over the bf16 baseline comes from fp8e4 DoubleRow matmuls with hi/lo
error compensation on all four projections:
  x ~= x_hi + x_lo (both fp8e4);  A@B ~= Ah@Bh + Al@Bh + Ah@Bl
Weights store pairs as (lo, hi), activations as (hi, lo), so
  - main terms pair two k-tiles per DoubleRow instruction:
      lhsT = (Wh[2k], Wh[2k+1]), rhs = (Xh[2k], Xh[2k+1])
  - correction terms pair the two cross products of one k-tile:
      lhsT = (Wl[kt], Wh[kt]),  rhs = (Xh[kt], Xl[kt])
Per k-tile this is 0.75x the bf16 PE-column cost, with accuracy slightly
better than bf16 (validated on device). hidden_states and all weights are
split host-side; the attention output is split on-chip.

Attention core (transposed scores, bf16) matches the baseline, except the
softmax denominators: instead of a ones-row matmul streaming the full P
tiles (N=QB columns each), the row-sums are computed transposed
(lhsT = P-tile, rhs = ones column -> out [q,1]), which costs ~1 PE cycle per
instruction. A PE transpose converts the [128,4] column sums to rows for the
reciprocal + gpsimd partition-broadcast normalization path.

Scales: hidden*16 and weights*64 keep fp8 splits in the normal range; the
inverses are folded into the rope tables, the V copy, and the final output
copy.  ao is scaled by 8 via the ones vector (memset 1/8) for the same
reason.
"""

import numpy as np
import ml_dtypes

import concourse.bacc as bacc
import concourse.bass as bass
import concourse.mybir as mybir
from concourse.tile import TileContext
from concourse import bass_utils

BF16 = mybir.dt.bfloat16
F8 = mybir.dt.float8e4
F32 = mybir.dt.float32
DR = mybir.MatmulPerfMode.DoubleRow

B, S, DIM = 2, 1024, 4096
T = B * S                     # 2048 tokens, batch-major
N_HEADS, N_KV = 32, 8
HD = 128                      # head dim == partition count
N_CORES = 8
HL = N_HEADS // N_CORES       # 4 local q-heads per core
KT = DIM // 128               # 32 feature tiles
CH = 512                      # projection token-chunk
NCHUNK = T // CH
QB = 512                      # attention q-block width
TT = T // 128                 # 16 token tiles global
SCALE = 1.0 / float(np.sqrt(HD))
SH = 16.0                     # hidden prescale for fp8
SW = 64.0                     # weight prescale for fp8
SA = 8.0                      # ao prescale (via ones = 1/8)

_CACHE: dict = {}


def _build():
    nc = bacc.Bacc("TRN2", target_bir_lowering=False, debug=False,
                   enable_asserts=False)

    # fp8 pair layouts: weights (lo, hi) on the pair axis, activations
    # (hi, lo).  Free layout of each weight dram row matches its SBUF tile.
    hs8 = nc.dram_tensor("hs8", [DIM, 2, T], F8, kind="ExternalInput")
    wq8 = nc.dram_tensor("wq8", [HL * 128, KT * 2 * HD], F8, kind="ExternalInput")
    wk8 = nc.dram_tensor("wk8", [128, KT * 2 * HD], F8, kind="ExternalInput")
    wv8 = nc.dram_tensor("wv8", [128, KT * 2 * HD], F8, kind="ExternalInput")
    wo8 = nc.dram_tensor("wo8", [HL * 128, 2 * DIM], F8, kind="ExternalInput")
    cos_q = nc.dram_tensor("cos_q", [HD, T], BF16, kind="ExternalInput")
    sin_q = nc.dram_tensor("sin_q", [HD, T], BF16, kind="ExternalInput")
    cos_k = nc.dram_tensor("cos_k", [HD, T], BF16, kind="ExternalInput")
    sin_k = nc.dram_tensor("sin_k", [HD, T], BF16, kind="ExternalInput")
    maskT = nc.dram_tensor("maskT", [128, 128], BF16, kind="ExternalInput")
    ident = nc.dram_tensor("ident", [128, 128], BF16, kind="ExternalInput")
    out = nc.dram_tensor("out", [T, DIM], BF16, kind="ExternalOutput")

    Exp = mybir.ActivationFunctionType.Exp

    with TileContext(nc) as tc:
        with tc.tile_pool(name="consts", bufs=1) as cpool, \
             tc.tile_pool(name="hs", bufs=2) as hpool, \
             tc.tile_pool(name="rope_tmp", bufs=2) as rpool, \
             tc.tile_pool(name="work_ps", bufs=2, space=bass.MemorySpace.PSUM) as wpool, \
             tc.tile_pool(name="ot_ps", bufs=2, space=bass.MemorySpace.PSUM) as otpool, \
             tc.tile_pool(name="rs_ps", bufs=1, space=bass.MemorySpace.PSUM) as rspool, \
             tc.tile_pool(name="et", bufs=3) as epool, \
             tc.tile_pool(name="rcp", bufs=2) as rcppool, \
             tc.tile_pool(name="rrow", bufs=2) as rrowpool, \
             tc.tile_pool(name="bc_sb", bufs=2) as bcsbpool, \
             tc.tile_pool(name="aobf", bufs=2) as aobfpool, \
             tc.tile_pool(name="out_sb", bufs=5) as xsbpool:

            wq_h = [cpool.tile([128, KT, 2, HD], F8, tag=f"wq{m}", name=f"wq{m}")
                    for m in range(HL)]
            wk_sb = cpool.tile([128, KT, 2, HD], F8, tag="wk")
            wv_sb = cpool.tile([128, KT, 2, HD], F8, tag="wv")
            cq_sb = cpool.tile([128, T], BF16, tag="cq")
            sq_sb = cpool.tile([128, T], BF16, tag="sq")
            ck_sb = cpool.tile([128, T], BF16, tag="ck")
            sk_sb = cpool.tile([128, T], BF16, tag="sk")
            maskT_sb = cpool.tile([128, 128], BF16, tag="maskT")
            ident_sb = cpool.tile([128, 128], BF16, tag="ident")
            ones_k = cpool.tile([128, 1], BF16, tag="ones_k")
            qt_rot = cpool.tile([128, HL, T], BF16, tag="qt")
            kt_rot = cpool.tile([128, T], BF16, tag="kt")
            v_sb = cpool.tile([128, TT, HD], BF16, tag="v")
            ao8 = cpool.tile([128, HL, 2, T], F8, tag="ao")

            hs_r = hs8.ap().rearrange("(kt p) two t -> p kt two t", p=128)

            # startup-critical DMA first: wq head 0 on the Act queue (the SP
            # queue starts streaming hs pieces in parallel)
            nc.sync.dma_start(wq_h[0][:, 0:8, :, :],
                              wq8.ap()[0:128, 0:8 * 2 * HD]
                              .rearrange("p (kt two n) -> p kt two n",
                                         two=2, n=HD))
            nc.sync.dma_start(wq_h[0][:, 8:KT, :, :],
                              wq8.ap()[0:128, 8 * 2 * HD:KT * 2 * HD]
                              .rearrange("p (kt two n) -> p kt two n",
                                         two=2, n=HD))

            def late_consts():
                # single SP queue, ordered by first use: the DMA device serves
                # one queue's backlog at a time, so explicit order beats a
                # second queue
                nc.sync.dma_start(
                    wq_h[1], wq8.ap()[128:256, :]
                    .rearrange("p (kt two n) -> p kt two n", two=2, n=HD))
                nc.sync.dma_start(cq_sb, cos_q.ap())
                nc.sync.dma_start(sq_sb, sin_q.ap())
                for m in range(2, HL):
                    nc.sync.dma_start(
                        wq_h[m], wq8.ap()[m * 128:(m + 1) * 128, :]
                        .rearrange("p (kt two n) -> p kt two n", two=2, n=HD))
                nc.sync.dma_start(
                    wk_sb, wk8.ap().rearrange("p (kt two n) -> p kt two n",
                                              two=2, n=HD))
                nc.sync.dma_start(ck_sb, cos_k.ap())
                nc.sync.dma_start(sk_sb, sin_k.ap())
                nc.sync.dma_start(
                    wv_sb, wv8.ap().rearrange("p (kt two n) -> p kt two n",
                                              two=2, n=HD))
                nc.sync.dma_start(maskT_sb, maskT.ap())
                nc.sync.dma_start(ident_sb, ident.ap())
                nc.vector.memset(ones_k, 1.0 / SA)

            def rope(ps, out_ap, cos_ap, sin_ap):
                """out = ps*cos + halfswap(ps)*sin  (signs baked into sin)."""
                t1 = rpool.tile([128, CH], F32, tag="r1", name="t1")
                t2 = rpool.tile([128, CH], F32, tag="r2", name="t2")
                nc.vector.tensor_mul(t1, ps, cos_ap)
                nc.vector.tensor_mul(t2[0:64, :], ps[64:128, :], sin_ap[0:64, :])
                nc.vector.tensor_mul(t2[64:128, :], ps[0:64, :], sin_ap[64:128, :])
                nc.vector.tensor_add(out_ap, t1, t2)

            def proj_mms(w_sb, hs_sb, vi=None):
                """fp8-compensated matmul operand pairs, emitted in 8-k-tile
                blocks (matching the hs DMA pieces, so chunk-0 compute streams
                with the loads).  vi selects the V orientation (hs
                stationary)."""
                mms = []
                for blk0 in range(0, KT, 8):
                    for k2 in range(blk0 // 2, blk0 // 2 + 4):
                        if vi is None:
                            mms.append((w_sb[:, 2 * k2:2 * k2 + 2, 1, :],
                                        hs_sb[:, 2 * k2:2 * k2 + 2, 0, :]))
                        else:
                            blk = slice(vi * 128, (vi + 1) * 128)
                            mms.append((hs_sb[:, 2 * k2:2 * k2 + 2, 0, blk],
                                        w_sb[:, 2 * k2:2 * k2 + 2, 1, :]))
                    for kt in range(blk0, blk0 + 8):
                        if vi is None:
                            mms.append((w_sb[:, kt, :, :], hs_sb[:, kt, :, :]))
                        else:
                            blk = slice(vi * 128, (vi + 1) * 128)
                            mms.append((hs_sb[:, kt, :, blk],
                                        w_sb[:, kt, :, :]))
                return mms

            # psum pair allocator: [128, 2, 512] f32 tiles (2 banks); Q/K/V
            # and O psums take halves, score tiles take whole pairs so one
            # double-width exp covers two k-tiles
            pair_cur = [None, 2]

            def half_ps(width=512):
                tile, j = pair_cur
                if j >= 2:
                    tile = wpool.tile([128, 2, 512], F32, tag="pair", name="pp")
                    j = 0
                pair_cur[0], pair_cur[1] = tile, j + 1
                return tile[:, j, 0:width]

            def reset_pairs():
                pair_cur[1] = 2

            def emit_q(c, m, hs_sb):
                t0 = c * CH
                ps = half_ps()
                mms = proj_mms(wq_h[m], hs_sb)
                for i, (lhsT, rhs) in enumerate(mms):
                    nc.tensor.matmul(ps, lhsT, rhs, start=(i == 0),
                                     stop=(i == len(mms) - 1), perf_mode=DR)
                if c == 0 and m == 0:
                    late_consts()
                rope(ps, qt_rot[:, m, t0:t0 + CH],
                     cq_sb[:, t0:t0 + CH], sq_sb[:, t0:t0 + CH])

            def emit_k(c, hs_sb):
                t0 = c * CH
                ps = half_ps()
                mms = proj_mms(wk_sb, hs_sb)
                for i, (lhsT, rhs) in enumerate(mms):
                    nc.tensor.matmul(ps, lhsT, rhs, start=(i == 0),
                                     stop=(i == len(mms) - 1), perf_mode=DR)
                rope(ps, kt_rot[:, t0:t0 + CH],
                     ck_sb[:, t0:t0 + CH], sk_sb[:, t0:t0 + CH])

            def emit_v(c, hs_sb):
                t0 = c * CH
                for vi in range(CH // 128):
                    tt = t0 // 128 + vi
                    ps = half_ps(HD)
                    mms = proj_mms(wv_sb, hs_sb, vi=vi)
                    for i, (lhsT, rhs) in enumerate(mms):
                        nc.tensor.matmul(ps, lhsT, rhs, start=(i == 0),
                                         stop=(i == len(mms) - 1), perf_mode=DR)
                    nc.scalar.mul(v_sb[:, tt, :], ps, 1.0 / (SH * SW))

            def emit_chunk(c):
                t0 = c * CH
                reset_pairs()
                hs_sb = hpool.tile([128, KT, 2, CH], F8, tag="hs", name="hs_sb")
                for g in range(4):
                    for s in range(2):
                        nc.sync.dma_start(
                            hs_sb[:, g * 8:(g + 1) * 8, s, :],
                            hs_r[:, g * 8:(g + 1) * 8, s, t0:t0 + CH])
                # Q0-Q2 stream with the hs/weight loads as they land; K/V
                # sit before Q3 so the K rope and v copies are done before
                # this chunk's attention groups need them
                emit_q(c, 0, hs_sb)
                emit_q(c, 1, hs_sb)
                emit_q(c, 2, hs_sb)
                emit_k(c, hs_sb)
                emit_v(c, hs_sb)
                emit_q(c, 3, hs_sb)

            # --- attention group machinery (transposed-scores scheme) ---
            pending = [None]

            def epilogue_rest(st):
                """Normalize ot by the per-token reciprocal row-sums and split
                the result into fp8 (hi, lo) for the O projection."""
                ot, h, q0, rrow = st
                bcs = bcsbpool.tile([128, QB], F32, tag="bcs", name="bcs")
                for j in range(4):
                    nc.gpsimd.partition_broadcast(bcs[:, j * 128:(j + 1) * 128],
                                                  rrow[0:1, :, j])
                abf = aobfpool.tile([128, QB], BF16, tag="abf", name="abf")
                nc.vector.tensor_mul(abf, ot, bcs)
                nc.vector.tensor_copy(ao8[:, h, 0, q0:q0 + QB], abf)
                nc.vector.tensor_sub(ao8[:, h, 1, q0:q0 + QB], abf,
                                     ao8[:, h, 0, q0:q0 + QB])

            def emit_group(b, h, qb):
                q0 = b * S + qb * QB
                n_kt = (qb + 1) * (QB // 128)
                n_pairs = n_kt // 2
                rs = rspool.tile([128, 4], F32, tag="rs", name="rs")
                ot = otpool.tile([128, QB], F32, tag="ot", name="ot")
                ets = [None] * n_kt

                def emit_sc_pair(p):
                    # both k-tiles of the pair score from the pair's smaller
                    # c0 (the extra non-causal columns are cheap and never
                    # consumed) so ONE double-width exp covers the pair
                    c0p = max(0, 2 * p - qb * (QB // 128)) * 128
                    scp = wpool.tile([128, 2, QB], F32, tag="pair", name="scp")
                    for j in range(2):
                        kt = 2 * p + j
                        jd = kt - qb * (QB // 128)
                        diag = 0 <= jd < QB // 128
                        nc.tensor.matmul(
                            scp[:, j, c0p:],
                            kt_rot[:, b * S + kt * 128:b * S + (kt + 1) * 128],
                            qt_rot[:, h, q0 + c0p:q0 + QB],
                            start=True, stop=not diag)
                        if diag:
                            # additive mask accumulated on the PE itself:
                            # keeps the score->exp->PV chain on one engine
                            nc.tensor.matmul(scp[:, j, jd * 128:(jd + 1) * 128],
                                             ident_sb, maskT_sb,
                                             start=False, stop=True,
                                             skip_group_check=True)
                    etp = epool.tile([128, 2, QB], BF16, tag="et", name="et")
                    nc.scalar.activation(etp[:, :, c0p:], scp[:, :, c0p:], Exp,
                                         bias=0.0, scale=1.0)
                    for j in range(2):
                        kt = 2 * p + j
                        c0 = max(0, kt - qb * (QB // 128)) * 128
                        ets[kt] = (etp, j, c0)

                for w in range(min(2, n_pairs)):
                    emit_sc_pair(w)
                for kt in range(n_kt):
                    if kt % 2 == 0 and kt // 2 + 2 < n_pairs:
                        emit_sc_pair(kt // 2 + 2)
                    etp, ej, c0 = ets[kt]
                    et = etp[:, ej, :]
                    # transposed row-sums: one near-free matmul per q-subblock.
                    # PSUM lazy-zeroing is bank-granular: exactly one
                    # start=True per group; later first-writes to other
                    # columns auto-zero via the pending-zero region.
                    for qs in range(c0 // 128, 4):
                        nc.tensor.matmul(rs[:, qs:qs + 1],
                                         et[:, qs * 128:(qs + 1) * 128], ones_k,
                                         start=(kt == 0 and qs == 0),
                                         stop=(kt == n_kt - 1 and qs == 3),
                                         skip_group_check=True)
                    nc.tensor.matmul(ot[:, c0:], v_sb[:, b * (S // 128) + kt, :],
                                     et[:, c0:], start=(kt == 0),
                                     stop=(kt == n_kt - 1))
                    ets[kt] = None
                    if kt == 2 and pending[0] is not None:
                        epilogue_rest(pending[0])
                        pending[0] = None
                # stage this group's epilogue: reciprocal of the [128, 4]
                # column sums, then a tiny DMA transposes them into a row
                # ([1, 128, 4]: token p of subblock j at free offset 4p+j);
                # the gpsimd broadcast + normalization run inside the next
                # group off that row.
                rcp = rcppool.tile([128, 4], F32, tag="rcp", name="rcp")
                nc.vector.reciprocal(rcp, rs)
                rrow = rrowpool.tile([1, 128, 4], F32, tag="rr", name="rrow")
                nc.sync.dma_start(rrow[0:1, :, :], rcp)
                pending[0] = (ot, h, q0, rrow)

            # --- interleaved emission: each chunk unblocks a set of groups ---
            wo_sb = None
            for c in range(NCHUNK):
                emit_chunk(c)
                b, qb = c // 2, c % 2
                for h in range(HL):
                    emit_group(b, h, qb)
                if c == NCHUNK - 1:
                    # wo reuses an hs slot (same size); DMA (Act queue, so the
                    # groups' rrow DMAs on SP aren't delayed) overlaps the
                    # final attention groups
                    wo_sb = hpool.tile([128, HL, 2, DIM], F8, tag="hs",
                                       name="wo_sb")
                    for m in range(HL):
                        nc.scalar.dma_start(wo_sb[:, m, :, :],
                                            wo8.ap()[m * 128:(m + 1) * 128, :]
                                            .rearrange("p (two n) -> p two n",
                                                       two=2))
            if pending[0] is not None:
                epilogue_rest(pending[0])
                pending[0] = None

            # ---- output projection (row-parallel Wo, fp8 compensated) ----
            # copies alternate Act/DVE into a double-width staging tile; one
            # out-DMA per two psum tiles keeps the SP queue + DGE fixed costs
            # off the critical chain
            reset_pairs()
            for tt in range(TT):
                tb = slice(tt * 128, (tt + 1) * 128)
                pairt = None
                for ni, n0 in enumerate(range(0, DIM, 512)):
                    ps = half_ps()
                    if ni % 2 == 0:
                        pairt = pair_cur[0]
                    mms = []
                    for j in range(HL // 2):
                        mms.append((ao8[:, 2 * j:2 * j + 2, 0, tb],
                                    wo_sb[:, 2 * j:2 * j + 2, 1, n0:n0 + 512]))
                    for kh in range(HL):
                        mms.append((ao8[:, kh, :, tb],
                                    wo_sb[:, kh, :, n0:n0 + 512]))
                    for i, (lhsT, rhs) in enumerate(mms):
                        nc.tensor.matmul(ps, lhsT, rhs, start=(i == 0),
                                         stop=(i == len(mms) - 1), perf_mode=DR)
                    if ni % 2 == 1:
                        # one double-width copy + DMA per psum pair
                        osb = xsbpool.tile([128, 1024], BF16, tag="osb",
                                           name="osb")
                        pview = pairt.rearrange("p two n -> p (two n)")
                        last = (tt == TT - 1) and (ni == 7)
                        if last:
                            # split the final store across engines + both
                            # HWDGE queues to shorten the end-of-kernel drain
                            nc.scalar.mul(osb[:, 0:512], pairt[:, 0, :],
                                          1.0 / (SW * SA))
                            nc.vector.tensor_scalar_mul(osb[:, 512:1024],
                                                        pairt[:, 1, :],
                                                        1.0 / (SW * SA))
                            nc.sync.dma_start(out.ap()[tb, n0 - 512:n0],
                                              osb[:, 0:512])
                            nc.scalar.dma_start(out.ap()[tb, n0:n0 + 512],
                                                osb[:, 512:1024])
                        else:
                            if (tt * 4 + ni // 2) % 2 == 0:
                                nc.scalar.mul(osb, pview, 1.0 / (SW * SA))
                            else:
                                nc.vector.tensor_scalar_mul(osb, pview,
                                                            1.0 / (SW * SA))
                            nc.sync.dma_start(out.ap()[tb, n0 - 512:n0 + 512],
                                              osb)
    nc.compile()
    return nc


def _get_nc():
    if "nc" not in _CACHE:
        _CACHE["nc"] = _build()
    return _CACHE["nc"]


def _split8(x):
    f8 = ml_dtypes.float8_e4m3
    hi = x.astype(f8)
    lo = (x - hi.astype(np.float32)).astype(f8)
    return hi, lo


def _prep_inputs(inputs) -> list[dict]:
    bf16 = ml_dtypes.bfloat16
    hs = np.asarray(inputs["hidden_states"], dtype=np.float32).reshape(T, DIM)
    hsT = np.ascontiguousarray(hs.T) * SH
    h_hi, h_lo = _split8(hsT)
    hs8 = np.ascontiguousarray(np.stack([h_hi, h_lo], axis=1))  # [DIM, 2, T]

    fc = np.asarray(inputs["freqs_cos"], dtype=np.float32).reshape(T, HD // 2).T
    fs = np.asarray(inputs["freqs_sin"], dtype=np.float32).reshape(T, HD // 2).T
    cos2 = np.concatenate([fc, fc], axis=0)            # [128, T]
    sin2 = np.concatenate([-fs, fs], axis=0)           # signed half-rotation
    qs = SCALE / (SH * SW)
    ks = 1.0 / (SH * SW)
    cos_qv = np.ascontiguousarray(cos2 * qs).astype(bf16)
    sin_qv = np.ascontiguousarray(sin2 * qs).astype(bf16)
    cos_kv = np.ascontiguousarray(cos2 * ks).astype(bf16)
    sin_kv = np.ascontiguousarray(sin2 * ks).astype(bf16)

    maskT = np.ascontiguousarray(
        np.asarray(inputs["attention_mask"], dtype=np.float32)[0, 0, :128, :128].T
    ).astype(ml_dtypes.bfloat16)
    ident = np.eye(128, dtype=np.float32).astype(ml_dtypes.bfloat16)

    perm = np.concatenate([np.arange(0, HD, 2), np.arange(1, HD, 2)])
    Wq = np.asarray(inputs["Wq"], dtype=np.float32)
    Wk = np.asarray(inputs["Wk"], dtype=np.float32)
    Wv = np.asarray(inputs["Wv"], dtype=np.float32)
    Wo = np.asarray(inputs["Wo"], dtype=np.float32)

    def pack_w(w, nheads):
        # w: [DIM, nheads*128] prescaled; -> [nheads*128p, KT*2*HD] with
        # (lo, hi) pairs: arr[m*128+p, ((kt*2)+s)*128+hd]
        hi, lo = _split8(w)
        pair = np.stack([lo.astype(np.float32), hi.astype(np.float32)], axis=1)
        # [DIM, 2, nheads*128] -> [KT, 128p, 2, nheads, 128hd]
        v = pair.reshape(KT, 128, 2, nheads, HD)
        arr = v.transpose(3, 1, 0, 2, 4).reshape(nheads * 128, KT * 2 * HD)
        return np.ascontiguousarray(arr).astype(ml_dtypes.float8_e4m3)

    in_maps = []
    for c in range(N_CORES):
        wq_c = np.concatenate(
            [Wq[:, (c * HL + h) * HD:(c * HL + h + 1) * HD][:, perm]
             for h in range(HL)], axis=1) * SW
        wk_c = Wk[:, c * HD:(c + 1) * HD][:, perm] * SW
        wv_c = Wv[:, c * HD:(c + 1) * HD] * SW
        wo_c = Wo[c * HL * HD:(c + 1) * HL * HD, :] * SW

        o_hi, o_lo = _split8(wo_c)
        wo_pack = np.concatenate([o_lo.astype(np.float32),
                                  o_hi.astype(np.float32)],
                                 axis=1)  # [512, 2*DIM] (lo block, hi block)
        wo_pack = np.ascontiguousarray(wo_pack).astype(ml_dtypes.float8_e4m3)

        in_maps.append({
            "hs8": hs8,
            "wq8": pack_w(wq_c, HL),
            "wk8": pack_w(wk_c, 1),
            "wv8": pack_w(wv_c, 1),
            "wo8": wo_pack,
            "cos_q": cos_qv, "sin_q": sin_qv,
            "cos_k": cos_kv, "sin_k": sin_kv,
            "maskT": maskT,
            "ident": ident,
        })
    return in_maps


def kernel(**inputs) -> np.ndarray:
    nc = _get_nc()
    in_maps = _prep_inputs(inputs)
    res = bass_utils.run_bass_kernel_spmd(nc, in_maps,
                                          core_ids=list(range(N_CORES)))
    acc = np.zeros((T, DIM), dtype=np.float32)
    for c in range(N_CORES):
        acc += np.asarray(res.results[c]["out"], dtype=np.float32)
    return acc.reshape(B, S, DIM)


# revision 51
# speedup vs baseline: 1.3342x; 1.0509x over previous
"""Trainium2 Bass kernel for MllamaTextSdpaAttention (GQA + RoPE + causal SDPA).

Tensor-parallel over heads across 8 NeuronCores (core c owns q-heads
[4c, 4c+4) and kv-head c); each core emits a partial [T, DIM] output that the
host sums in f32.

Speed over the bf16 baseline comes from fp8e4 DoubleRow matmuls with hi/lo
error compensation on all four projections:
  x ~= x_hi + x_lo (both fp8e4);  A@B ~= Ah@Bh + Al@Bh + Ah@Bl
Weights store pairs as (lo, hi), activations as (hi, lo), so
  - main terms pair two k-tiles per DoubleRow instruction:
      lhsT = (Wh[2k], Wh[2k+1]), rhs = (Xh[2k], Xh[2k+1])
  - correction terms pair the two cross products of one k-tile:
      lhsT = (Wl[kt], Wh[kt]),  rhs = (Xh[kt], Xl[kt])
Per k-tile this is 0.75x the bf16 PE-column cost, with accuracy slightly
better than bf16 (validated on device). hidden_states and all weights are
split host-side; the attention output is split on-chip.

Attention core (transposed scores, bf16) matches the baseline, except the
softmax denominators: instead of a ones-row matmul streaming the full P
tiles (N=QB columns each), the row-sums are computed transposed
(lhsT = P-tile, rhs = ones column -> out [q,1]), which costs ~1 PE cycle per
instruction. A PE transpose converts the [128,4] column sums to rows for the
reciprocal + gpsimd partition-broadcast normalization path.

Scales: hidden*16 and weights*64 keep fp8 splits in the normal range; the
inverses are folded into the rope tables, the V copy, and the final output
copy.  ao is scaled by 8 via the ones vector (memset 1/8) for the same
reason.
"""

import numpy as np
import ml_dtypes

import concourse.bacc as bacc
import concourse.bass as bass
import concourse.mybir as mybir
from concourse.tile import TileContext
from concourse import bass_utils

BF16 = mybir.dt.bfloat16
F8 = mybir.dt.float8e4
F32 = mybir.dt.float32
DR = mybir.MatmulPerfMode.DoubleRow

B, S, DIM = 2, 1024, 4096
T = B * S                     # 2048 tokens, batch-major
N_HEADS, N_KV = 32, 8
HD = 128                      # head dim == partition count
N_CORES = 8
HL = N_HEADS // N_CORES       # 4 local q-heads per core
KT = DIM // 128               # 32 feature tiles
CH = 512                      # projection token-chunk
NCHUNK = T // CH
QB = 512                      # attention q-block width
TT = T // 128                 # 16 token tiles global
SCALE = 1.0 / float(np.sqrt(HD))
SH = 16.0                     # hidden prescale for fp8
SW = 64.0                     # weight prescale for fp8
SA = 8.0                      # ao prescale (via ones = 1/8)
PREFETCH_UNIT = 8

_CACHE: dict = {}


def _build():
    nc = bacc.Bacc("TRN2", target_bir_lowering=False, debug=False,
                   enable_asserts=False)

    # fp8 pair layouts: weights (lo, hi) on the pair axis, activations
    # (hi, lo).  Free layout of each weight dram row matches its SBUF tile.
    hs8 = nc.dram_tensor("hs8", [DIM, 2, T], F8, kind="ExternalInput")
    wq8 = nc.dram_tensor("wq8", [HL * 128, KT * 2 * HD], F8, kind="ExternalInput")
    wk8 = nc.dram_tensor("wk8", [128, KT * 2 * HD], F8, kind="ExternalInput")
    wv8 = nc.dram_tensor("wv8", [128, KT * 2 * HD], F8, kind="ExternalInput")
    wo8 = nc.dram_tensor("wo8", [HL * 128, 2 * DIM], F8, kind="ExternalInput")
    cos_q = nc.dram_tensor("cos_q", [HD, T], BF16, kind="ExternalInput")
    sin_q = nc.dram_tensor("sin_q", [HD, T], BF16, kind="ExternalInput")
    cos_k = nc.dram_tensor("cos_k", [HD, T], BF16, kind="ExternalInput")
    sin_k = nc.dram_tensor("sin_k", [HD, T], BF16, kind="ExternalInput")
    maskT = nc.dram_tensor("maskT", [128, 128], BF16, kind="ExternalInput")
    ident = nc.dram_tensor("ident", [128, 128], BF16, kind="ExternalInput")
    out = nc.dram_tensor("out", [T, DIM], BF16, kind="ExternalOutput")

    Exp = mybir.ActivationFunctionType.Exp

    with TileContext(nc) as tc:
        with tc.tile_pool(name="consts", bufs=1) as cpool, \
             tc.tile_pool(name="hs", bufs=2) as hpool, \
             tc.tile_pool(name="rope_tmp", bufs=2) as rpool, \
             tc.tile_pool(name="work_ps", bufs=5, space=bass.MemorySpace.PSUM) as wpool, \
             tc.tile_pool(name="ot_ps", bufs=2, space=bass.MemorySpace.PSUM) as otpool, \
             tc.tile_pool(name="rs_ps", bufs=1, space=bass.MemorySpace.PSUM) as rspool, \
             tc.tile_pool(name="et", bufs=6) as epool, \
             tc.tile_pool(name="rcp", bufs=2) as rcppool, \
             tc.tile_pool(name="rrow", bufs=2) as rrowpool, \
             tc.tile_pool(name="bc_sb", bufs=2) as bcsbpool, \
             tc.tile_pool(name="aobf", bufs=2) as aobfpool, \
             tc.tile_pool(name="out_sb", bufs=5) as xsbpool:

            wq_h = [cpool.tile([128, KT, 2, HD], F8, tag=f"wq{m}", name=f"wq{m}")
                    for m in range(HL)]
            wk_sb = cpool.tile([128, KT, 2, HD], F8, tag="wk")
            wv_sb = cpool.tile([128, KT, 2, HD], F8, tag="wv")
            cq_sb = cpool.tile([128, T], BF16, tag="cq")
            sq_sb = cpool.tile([128, T], BF16, tag="sq")
            ck_sb = cpool.tile([128, T], BF16, tag="ck")
            sk_sb = cpool.tile([128, T], BF16, tag="sk")
            maskT_sb = cpool.tile([128, 128], BF16, tag="maskT")
            ones4 = cpool.tile([128, 4], F32, tag="ones4")
            ident_sb = cpool.tile([128, 128], BF16, tag="ident")
            ones_k = cpool.tile([128, 1], BF16, tag="ones_k")
            qt_rot = cpool.tile([128, HL, T], BF16, tag="qt")
            kt_rot = cpool.tile([128, T], BF16, tag="kt")
            v_sb = cpool.tile([128, TT, HD], BF16, tag="v")
            ao8 = cpool.tile([128, HL, 2, T], F8, tag="ao")

            hs_r = hs8.ap().rearrange("(kt p) two t -> p kt two t", p=128)

            # startup-critical DMA first: wq head 0 on the Act queue (the SP
            # queue starts streaming hs pieces in parallel)
            nc.sync.dma_start(wq_h[0][:, 0:8, :, :],
                              wq8.ap()[0:128, 0:8 * 2 * HD]
                              .rearrange("p (kt two n) -> p kt two n",
                                         two=2, n=HD))

            def late_consts():
                # single SP queue, ordered by first use: the DMA device serves
                # one queue's backlog at a time, so explicit order beats a
                # second queue
                nc.sync.dma_start(
                    wq_h[1], wq8.ap()[128:256, :]
                    .rearrange("p (kt two n) -> p kt two n", two=2, n=HD))
                nc.sync.dma_start(
                    wq_h[2], wq8.ap()[256:384, :]
                    .rearrange("p (kt two n) -> p kt two n", two=2, n=HD))
                nc.sync.dma_start(cq_sb, cos_q.ap())
                nc.sync.dma_start(sq_sb, sin_q.ap())
                nc.sync.dma_start(
                    wq_h[3], wq8.ap()[384:512, :]
                    .rearrange("p (kt two n) -> p kt two n", two=2, n=HD))
                nc.sync.dma_start(
                    wk_sb, wk8.ap().rearrange("p (kt two n) -> p kt two n",
                                              two=2, n=HD))
                nc.sync.dma_start(ck_sb, cos_k.ap())
                nc.sync.dma_start(sk_sb, sin_k.ap())
                nc.sync.dma_start(
                    wv_sb, wv8.ap().rearrange("p (kt two n) -> p kt two n",
                                              two=2, n=HD))
                nc.sync.dma_start(maskT_sb, maskT.ap())
                nc.sync.dma_start(ident_sb, ident.ap())
                nc.vector.memset(ones_k, 1.0 / SA)
                nc.vector.memset(ones4, 1.0)

            def rope(ps, out_ap, cos_ap, sin_ap):
                """out = ps*cos + halfswap(ps)*sin  (signs baked into sin)."""
                t1 = rpool.tile([128, CH], F32, tag="r1", name="t1")
                t2 = rpool.tile([128, CH], F32, tag="r2", name="t2")
                nc.vector.tensor_mul(t1, ps, cos_ap)
                nc.vector.tensor_mul(t2[0:64, :], ps[64:128, :], sin_ap[0:64, :])
                nc.vector.tensor_mul(t2[64:128, :], ps[0:64, :], sin_ap[64:128, :])
                nc.vector.tensor_add(out_ap, t1, t2)

            def proj_mms(w_sb, hs_sb, vi=None):
                """fp8-compensated matmul operand pairs, emitted in 8-k-tile
                blocks (matching the hs DMA pieces, so chunk-0 compute streams
                with the loads).  vi selects the V orientation (hs
                stationary)."""
                mms = []
                for blk0 in range(0, KT, 8):
                    for k2 in range(blk0 // 2, blk0 // 2 + 4):
                        if vi is None:
                            mms.append((w_sb[:, 2 * k2:2 * k2 + 2, 1, :],
                                        hs_sb[:, 2 * k2:2 * k2 + 2, 0, :]))
                        else:
                            blk = slice(vi * 128, (vi + 1) * 128)
                            mms.append((hs_sb[:, 2 * k2:2 * k2 + 2, 0, blk],
                                        w_sb[:, 2 * k2:2 * k2 + 2, 1, :]))
                    for kt in range(blk0, blk0 + 8):
                        if vi is None:
                            mms.append((w_sb[:, kt, :, :], hs_sb[:, kt, :, :]))
                        else:
                            blk = slice(vi * 128, (vi + 1) * 128)
                            mms.append((hs_sb[:, kt, :, blk],
                                        w_sb[:, kt, :, :]))
                return mms

            def gen_q(c, m, hs_sb):
                t0 = c * CH
                ps = wpool.tile([128, CH], F32, tag="work", name="ps_q")
                mms = proj_mms(wq_h[m], hs_sb)
                for i, (lhsT, rhs) in enumerate(mms):
                    nc.tensor.matmul(ps, lhsT, rhs, start=(i == 0),
                                     stop=(i == len(mms) - 1), perf_mode=DR)
                    if i % 12 == 11:
                        yield
                if c == 0 and m == 0:
                    late_consts()
                rope(ps, qt_rot[:, m, t0:t0 + CH],
                     cq_sb[:, t0:t0 + CH], sq_sb[:, t0:t0 + CH])
                yield

            def gen_k(c, hs_sb):
                t0 = c * CH
                ps = wpool.tile([128, CH], F32, tag="work", name="ps_k")
                mms = proj_mms(wk_sb, hs_sb)
                for i, (lhsT, rhs) in enumerate(mms):
                    nc.tensor.matmul(ps, lhsT, rhs, start=(i == 0),
                                     stop=(i == len(mms) - 1), perf_mode=DR)
                    if i % 12 == 11:
                        yield
                rope(ps, kt_rot[:, t0:t0 + CH],
                     ck_sb[:, t0:t0 + CH], sk_sb[:, t0:t0 + CH])
                yield

            def gen_v(c, hs_sb):
                t0 = c * CH
                for vi in range(CH // 128):
                    tt = t0 // 128 + vi
                    ps = wpool.tile([128, HD], F32, tag="work", name="ps_v")
                    mms = proj_mms(wv_sb, hs_sb, vi=vi)
                    for i, (lhsT, rhs) in enumerate(mms):
                        nc.tensor.matmul(ps, lhsT, rhs, start=(i == 0),
                                         stop=(i == len(mms) - 1), perf_mode=DR)
                        if i % 24 == 23:
                            yield
                    nc.scalar.mul(v_sb[:, tt, :], ps, 1.0 / (SH * SW))
                yield

            def gen_chunk(c, hs_sb):
                # Q0-Q2 stream with the hs/weight loads as they land; K/V
                # sit before Q3 so the K rope and v copies are done before
                # this chunk's attention groups need them
                yield from gen_q(c, 0, hs_sb)
                yield from gen_q(c, 1, hs_sb)
                yield from gen_q(c, 2, hs_sb)
                yield from gen_k(c, hs_sb)
                yield from gen_v(c, hs_sb)
                yield from gen_q(c, 3, hs_sb)

            # --- attention group machinery (transposed-scores scheme) ---
            pending = [None]

            def epilogue_rest(st):
                """Normalize ot by the per-token reciprocal row-sums and split
                the result into fp8 (hi, lo) for the O projection."""
                ot, h, q0, rrow = st
                bcs = bcsbpool.tile([128, QB], F32, tag="bcs", name="bcs")
                for j in range(4):
                    nc.gpsimd.partition_broadcast(bcs[:, j * 128:(j + 1) * 128],
                                                  rrow[0:1, :, j])
                abf = aobfpool.tile([128, QB], BF16, tag="abf", name="abf")
                nc.vector.tensor_mul(abf, ot, bcs)
                nc.vector.tensor_copy(ao8[:, h, 0, q0:q0 + QB], abf)
                nc.vector.tensor_sub(ao8[:, h, 1, q0:q0 + QB], abf,
                                     ao8[:, h, 0, q0:q0 + QB])

            def gen_group(b, h, qb):
                q0 = b * S + qb * QB
                n_kt = (qb + 1) * (QB // 128)
                rs = rspool.tile([128, 4], F32, tag="rs", name="rs")
                ot = otpool.tile([128, QB], F32, tag="ot", name="ot")
                ets = [None] * n_kt

                def emit_sc(kt):
                    c0 = max(0, kt - qb * (QB // 128)) * 128
                    jd = kt - qb * (QB // 128)
                    diag = 0 <= jd < QB // 128
                    sc = wpool.tile([128, QB], F32, tag="work", name="sc")
                    nc.tensor.matmul(
                        sc[:, c0:],
                        kt_rot[:, b * S + kt * 128:b * S + (kt + 1) * 128],
                        qt_rot[:, h, q0 + c0:q0 + QB],
                        start=True, stop=not diag)
                    if diag:
                        # additive mask accumulated on the PE itself
                        # (ident.T @ maskT): keeps the score->exp->PV chain
                        # on one engine instead of hopping DVE/Pool
                        nc.tensor.matmul(sc[:, jd * 128:(jd + 1) * 128],
                                         ident_sb, maskT_sb,
                                         start=False, stop=True,
                                         skip_group_check=True)
                    et = epool.tile([128, QB], BF16, tag="et", name="et")
                    nc.scalar.activation(et[:, c0:], sc[:, c0:], Exp,
                                         bias=0.0, scale=1.0)
                    ets[kt] = (et, c0)

                for w in range(min(5, n_kt)):
                    emit_sc(w)
                yield
                for kt in range(n_kt):
                    if kt + 5 < n_kt:
                        emit_sc(kt + 5)
                    et, c0 = ets[kt]
                    # transposed row-sums: one near-free matmul per q-subblock.
                    # PSUM lazy-zeroing is bank-granular: exactly one
                    # start=True per group; later first-writes to other
                    # columns auto-zero via the pending-zero region.
                    for qs in range(c0 // 128, 4):
                        nc.tensor.matmul(rs[:, qs:qs + 1],
                                         et[:, qs * 128:(qs + 1) * 128], ones_k,
                                         start=(kt == 0 and qs == 0),
                                         stop=(kt == n_kt - 1 and qs == 3),
                                         skip_group_check=True)
                    nc.tensor.matmul(ot[:, c0:], v_sb[:, b * (S // 128) + kt, :],
                                     et[:, c0:], start=(kt == 0),
                                     stop=(kt == n_kt - 1))
                    ets[kt] = None
                    if kt == 2 and pending[0] is not None:
                        epilogue_rest(pending[0])
                        pending[0] = None
                    yield
                # stage this group's epilogue: reciprocal of the [128, 4]
                # column sums, then a tiny DMA transposes them into a row
                # ([1, 128, 4]: token p of subblock j at free offset 4p+j);
                # the gpsimd broadcast + normalization run inside the next
                # group off that row.
                rcp = rcppool.tile([128, 4], F32, tag="rcp", name="rcp")
                nc.vector.reciprocal(rcp, rs)
                rrow = rrowpool.tile([1, 128, 4], F32, tag="rr", name="rrow")
                nc.sync.dma_start(rrow[0:1, :, :], rcp)
                pending[0] = (ot, h, q0, rrow)
                yield

            # ---- output projection tile (row-parallel Wo, fp8 comp.) ----
            # copies alternate Act/DVE into a double-width staging tile; one
            # out-DMA per two psum tiles keeps the SP queue + DGE fixed costs
            # off the critical chain
            wo_sb_box = [None]
            osb_box = [None]

            def emit_o_tile(tt, ni):
                wo_sb = wo_sb_box[0]
                n0 = ni * 512
                tb = slice(tt * 128, (tt + 1) * 128)
                ps = wpool.tile([128, 512], F32, tag="work", name="ps_o")
                mms = []
                for j in range(HL // 2):
                    mms.append((ao8[:, 2 * j:2 * j + 2, 0, tb],
                                wo_sb[:, 2 * j:2 * j + 2, 1, n0:n0 + 512]))
                for kh in range(HL):
                    mms.append((ao8[:, kh, :, tb],
                                wo_sb[:, kh, :, n0:n0 + 512]))
                for i, (lhsT, rhs) in enumerate(mms):
                    nc.tensor.matmul(ps, lhsT, rhs, start=(i == 0),
                                     stop=(i == len(mms) - 1), perf_mode=DR)
                if ni % 2 == 0:
                    osb_box[0] = xsbpool.tile([128, 1024], BF16, tag="osb",
                                              name="osb")
                osb = osb_box[0]
                half = osb[:, (ni % 2) * 512:(ni % 2) * 512 + 512]
                if (tt * 8 + ni) % 2 == 0:
                    nc.scalar.mul(half, ps, 1.0 / (SW * SA))
                else:
                    nc.vector.tensor_scalar_mul(half, ps, 1.0 / (SW * SA))
                if ni % 2 == 1:
                    last = (tt == TT - 1) and (ni == 7)
                    if last:
                        # split the final store across both HWDGE queues to
                        # shorten the end-of-kernel drain chain
                        nc.sync.dma_start(out.ap()[tb, n0 - 512:n0],
                                          osb[:, 0:512])
                        nc.scalar.dma_start(out.ap()[tb, n0:n0 + 512],
                                            osb[:, 512:1024])
                    else:
                        nc.sync.dma_start(out.ap()[tb, n0 - 512:n0 + 512],
                                          osb)

            # --- emission: chunk c's projection units round-robin with chunk
            # c-1's attention-group units (the Act exp stream smears over the
            # PE-heavy projection phases), and the last chunk's groups
            # interleave with the O projection ---
            def chain(gens):
                for g in gens:
                    yield from g

            def wq_dma(m):
                nc.sync.dma_start(
                    wq_h[m], wq8.ap()[m * 128:(m + 1) * 128, :]
                    .rearrange("p (kt two n) -> p kt two n", two=2, n=HD))

            def issue_hs0():
                """Chunk-0 loads, ordered so each arriving piece unblocks the
                next block-major unit of gen_chunk0."""
                hs_sb = hpool.tile([128, KT, 2, CH], F8, tag="hs",
                                   name="hs_sb")

                def hsp(b):
                    for g in (2 * b, 2 * b + 1):
                        for s in range(2):
                            nc.sync.dma_start(
                                hs_sb[:, g * 4:(g + 1) * 4, s, :],
                                hs_r[:, g * 4:(g + 1) * 4, s, 0:CH])

                hsp(0)
                nc.sync.dma_start(wq_h[0][:, 8:KT, :, :],
                                  wq8.ap()[0:128, 8 * 2 * HD:KT * 2 * HD]
                                  .rearrange("p (kt two n) -> p kt two n",
                                             two=2, n=HD))
                wq_dma(1)
                hsp(1)
                wq_dma(2)
                hsp(2)
                wq_dma(3)
                hsp(3)
                nc.sync.dma_start(
                    wk_sb, wk8.ap().rearrange("p (kt two n) -> p kt two n",
                                              two=2, n=HD))
                nc.sync.dma_start(cq_sb, cos_q.ap())
                nc.sync.dma_start(sq_sb, sin_q.ap())
                nc.sync.dma_start(ck_sb, cos_k.ap())
                nc.sync.dma_start(sk_sb, sin_k.ap())
                nc.sync.dma_start(
                    wv_sb, wv8.ap().rearrange("p (kt two n) -> p kt two n",
                                              two=2, n=HD))
                nc.sync.dma_start(maskT_sb, maskT.ap())
                nc.sync.dma_start(ident_sb, ident.ap())
                nc.vector.memset(ones_k, 1.0 / SA)
                nc.vector.memset(ones4, 1.0)
                return hs_sb

            def gen_chunk0(hs_sb):
                """Block-major chunk 0: five concurrent psum accumulations,
                8-k-tile blocks emitted in the order their DMA pieces land."""
                pss = [wpool.tile([128, CH], F32, tag="work", name=f"ps0_{j}")
                       for j in range(5)]
                projs = [proj_mms(wq_h[m], hs_sb) for m in range(HL)]
                projs.append(proj_mms(wk_sb, hs_sb))
                order = [(0, 0), (1, 0), (0, 1), (1, 1), (2, 0), (2, 1),
                         (0, 2), (1, 2), (2, 2), (3, 0), (3, 1), (3, 2),
                         (0, 3), (1, 3), (2, 3), (3, 3),
                         (4, 0), (4, 1), (4, 2), (4, 3)]
                for j, b in order:
                    for i in range(12 * b, 12 * b + 12):
                        lhsT, rhs = projs[j][i]
                        nc.tensor.matmul(pss[j], lhsT, rhs, start=(i == 0),
                                         stop=(i == 47), perf_mode=DR)
                    yield
                for m in range(HL):
                    rope(pss[m], qt_rot[:, m, 0:CH],
                         cq_sb[:, 0:CH], sq_sb[:, 0:CH])
                    yield
                rope(pss[4], kt_rot[:, 0:CH], ck_sb[:, 0:CH], sk_sb[:, 0:CH])
                yield
                yield from gen_v(0, hs_sb)

            def issue_hs(c):
                t0 = c * CH
                hs_sb = hpool.tile([128, KT, 2, CH], F8, tag="hs",
                                   name="hs_sb")
                for g in range(8):
                    for s in range(2):
                        nc.sync.dma_start(
                            hs_sb[:, g * 4:(g + 1) * 4, s, :],
                            hs_r[:, g * 4:(g + 1) * 4, s, t0:t0 + CH])
                return hs_sb

            prev_groups = []
            hs_cur = issue_hs0()
            for c in range(NCHUNK):
                cg = gen_chunk0(hs_cur) if c == 0 else gen_chunk(c, hs_cur)
                gg = chain([gen_group(*g) for g in prev_groups])
                # units: chunk ~33, groups 24 (qb0) / 40 (qb1)
                glen = (24 if (c - 1) % 2 == 0 else 40) if prev_groups else 0
                acc = 0.0
                qb1_groups = prev_groups and (c - 1) % 2 == 1
                unit = 0
                alive = True
                while alive:
                    alive = next(cg, StopIteration) is not StopIteration
                    unit += 1
                    if unit == PREFETCH_UNIT and c + 1 < NCHUNK:
                        # prefetch next chunk's hs mid-chunk so its first
                        # pieces land before the next projections start
                        hs_cur = issue_hs(c + 1)
                    acc += glen / 33.0
                    while acc >= 1.0:
                        next(gg, None)
                        acc -= 1.0
                for _ in gg:
                    pass
                b, qb = c // 2, c % 2
                prev_groups = [(b, h, qb) for h in range(HL)]
                if c == NCHUNK - 1:
                    # wo reuses an hs slot (same size); DMA on the Act queue
                    # overlaps the final groups
                    wo_sb_box[0] = hpool.tile([128, HL, 2, DIM], F8, tag="hs",
                                              name="wo_sb")
                    for m in range(HL):
                        nc.scalar.dma_start(
                            wo_sb_box[0][:, m, :, :],
                            wo8.ap()[m * 128:(m + 1) * 128, :]
                            .rearrange("p (two n) -> p two n", two=2))

            # tail: last chunk's groups interleaved with O tiles of the
            # earlier chunks, then the rest of the O projection
            o_list = [(tt, ni) for tt in range(TT) for ni in range(8)]
            o_head = 0
            gg = chain([gen_group(*g) for g in prev_groups])
            oacc = 0.0
            for _ in gg:
                oacc += 2.4
                while oacc >= 1.0 and o_head < 96:
                    emit_o_tile(*o_list[o_head])
                    o_head += 1
                    oacc -= 1.0
            if pending[0] is not None:
                epilogue_rest(pending[0])
                pending[0] = None
            while o_head < len(o_list):
                emit_o_tile(*o_list[o_head])
                o_head += 1
    nc.compile()
    return nc


def _get_nc():
    if "nc" not in _CACHE:
        _CACHE["nc"] = _build()
    return _CACHE["nc"]


def _split8(x):
    f8 = ml_dtypes.float8_e4m3
    hi = x.astype(f8)
    lo = (x - hi.astype(np.float32)).astype(f8)
    return hi, lo


def _prep_inputs(inputs) -> list[dict]:
    bf16 = ml_dtypes.bfloat16
    hs = np.asarray(inputs["hidden_states"], dtype=np.float32).reshape(T, DIM)
    hsT = np.ascontiguousarray(hs.T) * SH
    h_hi, h_lo = _split8(hsT)
    hs8 = np.ascontiguousarray(np.stack([h_hi, h_lo], axis=1))  # [DIM, 2, T]

    fc = np.asarray(inputs["freqs_cos"], dtype=np.float32).reshape(T, HD // 2).T
    fs = np.asarray(inputs["freqs_sin"], dtype=np.float32).reshape(T, HD // 2).T
    cos2 = np.concatenate([fc, fc], axis=0)            # [128, T]
    sin2 = np.concatenate([-fs, fs], axis=0)           # signed half-rotation
    qs = SCALE / (SH * SW)
    ks = 1.0 / (SH * SW)
    cos_qv = np.ascontiguousarray(cos2 * qs).astype(bf16)
    sin_qv = np.ascontiguousarray(sin2 * qs).astype(bf16)
    cos_kv = np.ascontiguousarray(cos2 * ks).astype(bf16)
    sin_kv = np.ascontiguousarray(sin2 * ks).astype(bf16)

    maskT = np.ascontiguousarray(
        np.asarray(inputs["attention_mask"], dtype=np.float32)[0, 0, :128, :128].T
    ).astype(ml_dtypes.bfloat16)
    ident = np.eye(128, dtype=np.float32).astype(ml_dtypes.bfloat16)

    perm = np.concatenate([np.arange(0, HD, 2), np.arange(1, HD, 2)])
    Wq = np.asarray(inputs["Wq"], dtype=np.float32)
    Wk = np.asarray(inputs["Wk"], dtype=np.float32)
    Wv = np.asarray(inputs["Wv"], dtype=np.float32)
    Wo = np.asarray(inputs["Wo"], dtype=np.float32)

    def pack_w(w, nheads):
        # w: [DIM, nheads*128] prescaled; -> [nheads*128p, KT*2*HD] with
        # (lo, hi) pairs: arr[m*128+p, ((kt*2)+s)*128+hd]
        hi, lo = _split8(w)
        pair = np.stack([lo.astype(np.float32), hi.astype(np.float32)], axis=1)
        # [DIM, 2, nheads*128] -> [KT, 128p, 2, nheads, 128hd]
        v = pair.reshape(KT, 128, 2, nheads, HD)
        arr = v.transpose(3, 1, 0, 2, 4).reshape(nheads * 128, KT * 2 * HD)
        return np.ascontiguousarray(arr).astype(ml_dtypes.float8_e4m3)

    in_maps = []
    for c in range(N_CORES):
        wq_c = np.concatenate(
            [Wq[:, (c * HL + h) * HD:(c * HL + h + 1) * HD][:, perm]
             for h in range(HL)], axis=1) * SW
        wk_c = Wk[:, c * HD:(c + 1) * HD][:, perm] * SW
        wv_c = Wv[:, c * HD:(c + 1) * HD] * SW
        wo_c = Wo[c * HL * HD:(c + 1) * HL * HD, :] * SW

        o_hi, o_lo = _split8(wo_c)
        wo_pack = np.concatenate([o_lo.astype(np.float32),
                                  o_hi.astype(np.float32)],
                                 axis=1)  # [512, 2*DIM] (lo block, hi block)
        wo_pack = np.ascontiguousarray(wo_pack).astype(ml_dtypes.float8_e4m3)

        in_maps.append({
            "hs8": hs8,
            "wq8": pack_w(wq_c, HL),
            "wk8": pack_w(wk_c, 1),
            "wv8": pack_w(wv_c, 1),
            "wo8": wo_pack,
            "cos_q": cos_qv, "sin_q": sin_qv,
            "cos_k": cos_kv, "sin_k": sin_kv,
            "maskT": maskT,
            "ident": ident,
        })
    return in_maps


def kernel(**inputs) -> np.ndarray:
    nc = _get_nc()
    in_maps = _prep_inputs(inputs)
    res = bass_utils.run_bass_kernel_spmd(nc, in_maps,
                                          core_ids=list(range(N_CORES)))
    acc = np.zeros((T, DIM), dtype=np.float32)
    for c in range(N_CORES):
        acc += np.asarray(res.results[c]["out"], dtype=np.float32)
    return acc.reshape(B, S, DIM)


# revision 52
# speedup vs baseline: 1.3508x; 1.0125x over previous
"""Trainium2 Bass kernel for MllamaTextSdpaAttention (GQA + RoPE + causal SDPA).

Tensor-parallel over heads across 8 NeuronCores (core c owns q-heads
[4c, 4c+4) and kv-head c); each core emits a partial [T, DIM] output that the
host sums in f32.

Speed over the bf16 baseline comes from fp8e4 DoubleRow matmuls with hi/lo
error compensation on all four projections:
  x ~= x_hi + x_lo (both fp8e4);  A@B ~= Ah@Bh + Al@Bh + Ah@Bl
Weights store pairs as (lo, hi), activations as (hi, lo), so
  - main terms pair two k-tiles per DoubleRow instruction:
      lhsT = (Wh[2k], Wh[2k+1]), rhs = (Xh[2k], Xh[2k+1])
  - correction terms pair the two cross products of one k-tile:
      lhsT = (Wl[kt], Wh[kt]),  rhs = (Xh[kt], Xl[kt])
Per k-tile this is 0.75x the bf16 PE-column cost, with accuracy slightly
better than bf16 (validated on device). hidden_states and all weights are
split host-side; the attention output is split on-chip.

Attention core (transposed scores, bf16) matches the baseline, except the
softmax denominators: instead of a ones-row matmul streaming the full P
tiles (N=QB columns each), the row-sums are computed transposed
(lhsT = P-tile, rhs = ones column -> out [q,1]), which costs ~1 PE cycle per
instruction. A PE transpose converts the [128,4] column sums to rows for the
reciprocal + gpsimd partition-broadcast normalization path.

Scales: hidden*16 and weights*64 keep fp8 splits in the normal range; the
inverses are folded into the rope tables, the V copy, and the final output
copy.  ao is scaled by 8 via the ones vector (memset 1/8) for the same
reason.
"""

import numpy as np
import ml_dtypes

import concourse.bacc as bacc
import concourse.bass as bass
import concourse.mybir as mybir
from concourse.tile import TileContext
from concourse import bass_utils

BF16 = mybir.dt.bfloat16
F8 = mybir.dt.float8e4
F32 = mybir.dt.float32
DR = mybir.MatmulPerfMode.DoubleRow

B, S, DIM = 2, 1024, 4096
T = B * S                     # 2048 tokens, batch-major
N_HEADS, N_KV = 32, 8
HD = 128                      # head dim == partition count
N_CORES = 8
HL = N_HEADS // N_CORES       # 4 local q-heads per core
KT = DIM // 128               # 32 feature tiles
CH = 512                      # projection token-chunk
NCHUNK = T // CH
QB = 512                      # attention q-block width
TT = T // 128                 # 16 token tiles global
SCALE = 1.0 / float(np.sqrt(HD))
SH = 16.0                     # hidden prescale for fp8
SW = 64.0                     # weight prescale for fp8
SA = 8.0                      # ao prescale (via ones = 1/8)
PREFETCH_UNIT = 8

_CACHE: dict = {}


def _build():
    nc = bacc.Bacc("TRN2", target_bir_lowering=False, debug=False,
                   enable_asserts=False)

    # fp8 pair layouts: weights (lo, hi) on the pair axis, activations
    # (hi, lo).  Free layout of each weight dram row matches its SBUF tile.
    hs8 = nc.dram_tensor("hs8", [DIM, 2, T], F8, kind="ExternalInput")
    wq8 = nc.dram_tensor("wq8", [HL * 128, KT * 2 * HD], F8, kind="ExternalInput")
    wk8 = nc.dram_tensor("wk8", [128, KT * 2 * HD], F8, kind="ExternalInput")
    wv8 = nc.dram_tensor("wv8", [128, KT * 2 * HD], F8, kind="ExternalInput")
    wo8 = nc.dram_tensor("wo8", [HL * 128, 2 * DIM], F8, kind="ExternalInput")
    cos_q = nc.dram_tensor("cos_q", [HD, T], BF16, kind="ExternalInput")
    sin_q = nc.dram_tensor("sin_q", [HD, T], BF16, kind="ExternalInput")
    cos_k = nc.dram_tensor("cos_k", [HD, T], BF16, kind="ExternalInput")
    sin_k = nc.dram_tensor("sin_k", [HD, T], BF16, kind="ExternalInput")
    maskT = nc.dram_tensor("maskT", [128, 128], BF16, kind="ExternalInput")
    ident = nc.dram_tensor("ident", [128, 128], BF16, kind="ExternalInput")
    out = nc.dram_tensor("out", [T, DIM], BF16, kind="ExternalOutput")

    Exp = mybir.ActivationFunctionType.Exp

    with TileContext(nc) as tc:
        with tc.tile_pool(name="consts", bufs=1) as cpool, \
             tc.tile_pool(name="hs", bufs=2) as hpool, \
             tc.tile_pool(name="rope_tmp", bufs=2) as rpool, \
             tc.tile_pool(name="work_ps", bufs=5, space=bass.MemorySpace.PSUM) as wpool, \
             tc.tile_pool(name="ot_ps", bufs=2, space=bass.MemorySpace.PSUM) as otpool, \
             tc.tile_pool(name="rs_ps", bufs=1, space=bass.MemorySpace.PSUM) as rspool, \
             tc.tile_pool(name="et", bufs=6) as epool, \
             tc.tile_pool(name="rcp", bufs=2) as rcppool, \
             tc.tile_pool(name="rrow", bufs=2) as rrowpool, \
             tc.tile_pool(name="bc_sb", bufs=2) as bcsbpool, \
             tc.tile_pool(name="aobf", bufs=2) as aobfpool, \
             tc.tile_pool(name="out_sb", bufs=5) as xsbpool:

            wq_h = [cpool.tile([128, KT, 2, HD], F8, tag=f"wq{m}", name=f"wq{m}")
                    for m in range(HL)]
            wk_sb = cpool.tile([128, KT, 2, HD], F8, tag="wk")
            wv_sb = cpool.tile([128, KT, 2, HD], F8, tag="wv")
            cq_sb = cpool.tile([128, T], BF16, tag="cq")
            sq_sb = cpool.tile([128, T], BF16, tag="sq")
            ck_sb = cpool.tile([128, T], BF16, tag="ck")
            sk_sb = cpool.tile([128, T], BF16, tag="sk")
            maskT_sb = cpool.tile([128, 128], BF16, tag="maskT")
            ones4 = cpool.tile([128, 4], F32, tag="ones4")
            ident_sb = cpool.tile([128, 128], BF16, tag="ident")
            ones_k = cpool.tile([128, 1], BF16, tag="ones_k")
            qt_rot = cpool.tile([128, HL, T], BF16, tag="qt")
            kt_rot = cpool.tile([128, T], BF16, tag="kt")
            v_sb = cpool.tile([128, TT, HD], BF16, tag="v")
            ao8 = cpool.tile([128, HL, 2, T], F8, tag="ao")

            hs_r = hs8.ap().rearrange("(kt p) two t -> p kt two t", p=128)

            # startup-critical DMA first: wq head 0 on the Act queue (the SP
            # queue starts streaming hs pieces in parallel)
            nc.sync.dma_start(wq_h[0][:, 0:8, :, :],
                              wq8.ap()[0:128, 0:8 * 2 * HD]
                              .rearrange("p (kt two n) -> p kt two n",
                                         two=2, n=HD))

            def late_consts():
                # single SP queue, ordered by first use: the DMA device serves
                # one queue's backlog at a time, so explicit order beats a
                # second queue
                nc.sync.dma_start(
                    wq_h[1], wq8.ap()[128:256, :]
                    .rearrange("p (kt two n) -> p kt two n", two=2, n=HD))
                nc.sync.dma_start(
                    wq_h[2], wq8.ap()[256:384, :]
                    .rearrange("p (kt two n) -> p kt two n", two=2, n=HD))
                nc.sync.dma_start(cq_sb, cos_q.ap())
                nc.sync.dma_start(sq_sb, sin_q.ap())
                nc.sync.dma_start(
                    wq_h[3], wq8.ap()[384:512, :]
                    .rearrange("p (kt two n) -> p kt two n", two=2, n=HD))
                nc.sync.dma_start(
                    wk_sb, wk8.ap().rearrange("p (kt two n) -> p kt two n",
                                              two=2, n=HD))
                nc.sync.dma_start(ck_sb, cos_k.ap())
                nc.sync.dma_start(sk_sb, sin_k.ap())
                nc.sync.dma_start(
                    wv_sb, wv8.ap().rearrange("p (kt two n) -> p kt two n",
                                              two=2, n=HD))
                nc.sync.dma_start(maskT_sb, maskT.ap())
                nc.sync.dma_start(ident_sb, ident.ap())
                nc.vector.memset(ones_k, 1.0 / SA)
                nc.vector.memset(ones4, 1.0)

            def rope(ps, out_ap, cos_ap, sin_ap):
                """out = ps*cos + halfswap(ps)*sin  (signs baked into sin)."""
                t1 = rpool.tile([128, CH], F32, tag="r1", name="t1")
                t2 = rpool.tile([128, CH], F32, tag="r2", name="t2")
                nc.vector.tensor_mul(t1, ps, cos_ap)
                nc.vector.tensor_mul(t2[0:64, :], ps[64:128, :], sin_ap[0:64, :])
                nc.vector.tensor_mul(t2[64:128, :], ps[0:64, :], sin_ap[64:128, :])
                nc.vector.tensor_add(out_ap, t1, t2)

            def proj_mms(w_sb, hs_sb, vi=None):
                """fp8-compensated matmul operand pairs, emitted in 8-k-tile
                blocks (matching the hs DMA pieces, so chunk-0 compute streams
                with the loads).  vi selects the V orientation (hs
                stationary)."""
                mms = []
                for blk0 in range(0, KT, 8):
                    for k2 in range(blk0 // 2, blk0 // 2 + 4):
                        if vi is None:
                            mms.append((w_sb[:, 2 * k2:2 * k2 + 2, 1, :],
                                        hs_sb[:, 2 * k2:2 * k2 + 2, 0, :]))
                        else:
                            blk = slice(vi * 128, (vi + 1) * 128)
                            mms.append((hs_sb[:, 2 * k2:2 * k2 + 2, 0, blk],
                                        w_sb[:, 2 * k2:2 * k2 + 2, 1, :]))
                    for kt in range(blk0, blk0 + 8):
                        if vi is None:
                            mms.append((w_sb[:, kt, :, :], hs_sb[:, kt, :, :]))
                        else:
                            blk = slice(vi * 128, (vi + 1) * 128)
                            mms.append((hs_sb[:, kt, :, blk],
                                        w_sb[:, kt, :, :]))
                return mms

            def gen_q(c, m, hs_sb):
                t0 = c * CH
                ps = wpool.tile([128, CH], F32, tag="work", name="ps_q")
                mms = proj_mms(wq_h[m], hs_sb)
                for i, (lhsT, rhs) in enumerate(mms):
                    nc.tensor.matmul(ps, lhsT, rhs, start=(i == 0),
                                     stop=(i == len(mms) - 1), perf_mode=DR)
                    if i % 12 == 11:
                        yield
                if c == 0 and m == 0:
                    late_consts()
                rope(ps, qt_rot[:, m, t0:t0 + CH],
                     cq_sb[:, t0:t0 + CH], sq_sb[:, t0:t0 + CH])
                yield

            def gen_k(c, hs_sb):
                t0 = c * CH
                ps = wpool.tile([128, CH], F32, tag="work", name="ps_k")
                mms = proj_mms(wk_sb, hs_sb)
                for i, (lhsT, rhs) in enumerate(mms):
                    nc.tensor.matmul(ps, lhsT, rhs, start=(i == 0),
                                     stop=(i == len(mms) - 1), perf_mode=DR)
                    if i % 12 == 11:
                        yield
                rope(ps, kt_rot[:, t0:t0 + CH],
                     ck_sb[:, t0:t0 + CH], sk_sb[:, t0:t0 + CH])
                yield

            def gen_v(c, hs_sb):
                t0 = c * CH
                for vi in range(CH // 128):
                    tt = t0 // 128 + vi
                    ps = wpool.tile([128, HD], F32, tag="work", name="ps_v")
                    mms = proj_mms(wv_sb, hs_sb, vi=vi)
                    for i, (lhsT, rhs) in enumerate(mms):
                        nc.tensor.matmul(ps, lhsT, rhs, start=(i == 0),
                                         stop=(i == len(mms) - 1), perf_mode=DR)
                        if i % 24 == 23:
                            yield
                    nc.scalar.mul(v_sb[:, tt, :], ps, 1.0 / (SH * SW))
                yield

            def gen_chunk(c, hs_sb):
                # Q0-Q2 stream with the hs/weight loads as they land; K/V
                # sit before Q3 so the K rope and v copies are done before
                # this chunk's attention groups need them
                yield from gen_q(c, 0, hs_sb)
                yield from gen_q(c, 1, hs_sb)
                yield from gen_q(c, 2, hs_sb)
                yield from gen_k(c, hs_sb)
                yield from gen_v(c, hs_sb)
                yield from gen_q(c, 3, hs_sb)

            # --- attention group machinery (transposed-scores scheme) ---
            pending = [None]

            def epilogue_rest(st):
                """Normalize ot by the per-token reciprocal row-sums and split
                the result into fp8 (hi, lo) for the O projection."""
                ot, h, q0, rrow = st
                bcs = bcsbpool.tile([128, QB], F32, tag="bcs", name="bcs")
                for j in range(4):
                    nc.gpsimd.partition_broadcast(bcs[:, j * 128:(j + 1) * 128],
                                                  rrow[0:1, :, j])
                abf = aobfpool.tile([128, QB], BF16, tag="abf", name="abf")
                nc.vector.tensor_mul(abf, ot, bcs)
                nc.vector.tensor_copy(ao8[:, h, 0, q0:q0 + QB], abf)
                nc.vector.tensor_sub(ao8[:, h, 1, q0:q0 + QB], abf,
                                     ao8[:, h, 0, q0:q0 + QB])

            def gen_group(b, h, qb):
                q0 = b * S + qb * QB
                n_kt = (qb + 1) * (QB // 128)
                rs = rspool.tile([128, 4], F32, tag="rs", name="rs")
                ot = otpool.tile([128, QB], F32, tag="ot", name="ot")
                ets = [None] * n_kt

                def emit_sc(kt):
                    c0 = max(0, kt - qb * (QB // 128)) * 128
                    jd = kt - qb * (QB // 128)
                    diag = 0 <= jd < QB // 128
                    sc = wpool.tile([128, QB], F32, tag="work", name="sc")
                    nc.tensor.matmul(
                        sc[:, c0:],
                        kt_rot[:, b * S + kt * 128:b * S + (kt + 1) * 128],
                        qt_rot[:, h, q0 + c0:q0 + QB],
                        start=True, stop=not diag)
                    if diag:
                        # additive mask accumulated on the PE itself
                        # (ident.T @ maskT): keeps the score->exp->PV chain
                        # on one engine instead of hopping DVE/Pool
                        nc.tensor.matmul(sc[:, jd * 128:(jd + 1) * 128],
                                         ident_sb, maskT_sb,
                                         start=False, stop=True,
                                         skip_group_check=True)
                    et = epool.tile([128, QB], BF16, tag="et", name="et")
                    nc.scalar.activation(et[:, c0:], sc[:, c0:], Exp,
                                         bias=0.0, scale=1.0)
                    ets[kt] = (et, c0)

                for w in range(min(5, n_kt)):
                    emit_sc(w)
                yield
                for kt in range(n_kt):
                    if kt + 5 < n_kt:
                        emit_sc(kt + 5)
                    et, c0 = ets[kt]
                    # transposed row-sums: one near-free matmul per q-subblock.
                    # PSUM lazy-zeroing is bank-granular: exactly one
                    # start=True per group; later first-writes to other
                    # columns auto-zero via the pending-zero region.
                    for qs in range(c0 // 128, 4):
                        nc.tensor.matmul(rs[:, qs:qs + 1],
                                         et[:, qs * 128:(qs + 1) * 128], ones_k,
                                         start=(kt == 0 and qs == 0),
                                         stop=(kt == n_kt - 1 and qs == 3),
                                         skip_group_check=True)
                    nc.tensor.matmul(ot[:, c0:], v_sb[:, b * (S // 128) + kt, :],
                                     et[:, c0:], start=(kt == 0),
                                     stop=(kt == n_kt - 1))
                    ets[kt] = None
                    if kt == 2 and pending[0] is not None:
                        epilogue_rest(pending[0])
                        pending[0] = None
                    yield
                # stage this group's epilogue: reciprocal of the [128, 4]
                # column sums, then a tiny DMA transposes them into a row
                # ([1, 128, 4]: token p of subblock j at free offset 4p+j);
                # the gpsimd broadcast + normalization run inside the next
                # group off that row.
                rcp = rcppool.tile([128, 4], F32, tag="rcp", name="rcp")
                nc.vector.reciprocal(rcp, rs)
                rrow = rrowpool.tile([1, 128, 4], F32, tag="rr", name="rrow")
                nc.sync.dma_start(rrow[0:1, :, :], rcp)
                pending[0] = (ot, h, q0, rrow)
                yield

            # ---- output projection tile (row-parallel Wo, fp8 comp.) ----
            # copies alternate Act/DVE into a double-width staging tile; one
            # out-DMA per two psum tiles keeps the SP queue + DGE fixed costs
            # off the critical chain
            wo_sb_box = [None]
            osb_box = [None]

            def emit_o_tile(tt, ni):
                wo_sb = wo_sb_box[0]
                n0 = ni * 512
                tb = slice(tt * 128, (tt + 1) * 128)
                ps = wpool.tile([128, 512], F32, tag="work", name="ps_o")
                mms = []
                for j in range(HL // 2):
                    mms.append((ao8[:, 2 * j:2 * j + 2, 0, tb],
                                wo_sb[:, 2 * j:2 * j + 2, 1, n0:n0 + 512]))
                for kh in range(HL):
                    mms.append((ao8[:, kh, :, tb],
                                wo_sb[:, kh, :, n0:n0 + 512]))
                for i, (lhsT, rhs) in enumerate(mms):
                    nc.tensor.matmul(ps, lhsT, rhs, start=(i == 0),
                                     stop=(i == len(mms) - 1), perf_mode=DR)
                if ni % 2 == 0:
                    osb_box[0] = xsbpool.tile([128, 1024], BF16, tag="osb",
                                              name="osb")
                osb = osb_box[0]
                half = osb[:, (ni % 2) * 512:(ni % 2) * 512 + 512]
                if (tt * 8 + ni) % 2 == 0:
                    nc.scalar.mul(half, ps, 1.0 / (SW * SA))
                else:
                    nc.vector.tensor_scalar_mul(half, ps, 1.0 / (SW * SA))
                if ni % 2 == 1:
                    last = (tt == TT - 1) and (ni == 7)
                    if last:
                        # split the final store across both HWDGE queues to
                        # shorten the end-of-kernel drain chain
                        nc.sync.dma_start(out.ap()[tb, n0 - 512:n0],
                                          osb[:, 0:512])
                        nc.scalar.dma_start(out.ap()[tb, n0:n0 + 512],
                                            osb[:, 512:1024])
                    else:
                        nc.sync.dma_start(out.ap()[tb, n0 - 512:n0 + 512],
                                          osb)

            # --- emission: chunk c's projection units round-robin with chunk
            # c-1's attention-group units (the Act exp stream smears over the
            # PE-heavy projection phases), and the last chunk's groups
            # interleave with the O projection ---
            def chain(gens):
                for g in gens:
                    yield from g

            def wq_dma(m):
                nc.sync.dma_start(
                    wq_h[m], wq8.ap()[m * 128:(m + 1) * 128, :]
                    .rearrange("p (kt two n) -> p kt two n", two=2, n=HD))

            def issue_hs0():
                """Chunk-0 loads, ordered so each arriving piece unblocks the
                next block-major unit of gen_chunk0."""
                hs_sb = hpool.tile([128, KT, 2, CH], F8, tag="hs",
                                   name="hs_sb")

                def hsp(b):
                    for s in range(2):
                        nc.sync.dma_start(
                            hs_sb[:, b * 8:(b + 1) * 8, s, :],
                            hs_r[:, b * 8:(b + 1) * 8, s, 0:CH])

                hsp(0)
                nc.sync.dma_start(wq_h[0][:, 8:KT, :, :],
                                  wq8.ap()[0:128, 8 * 2 * HD:KT * 2 * HD]
                                  .rearrange("p (kt two n) -> p kt two n",
                                             two=2, n=HD))
                wq_dma(1)
                hsp(1)
                wq_dma(2)
                hsp(2)
                wq_dma(3)
                hsp(3)
                nc.sync.dma_start(
                    wk_sb, wk8.ap().rearrange("p (kt two n) -> p kt two n",
                                              two=2, n=HD))
                nc.sync.dma_start(cq_sb, cos_q.ap())
                nc.sync.dma_start(sq_sb, sin_q.ap())
                nc.sync.dma_start(ck_sb, cos_k.ap())
                nc.sync.dma_start(sk_sb, sin_k.ap())
                nc.sync.dma_start(
                    wv_sb, wv8.ap().rearrange("p (kt two n) -> p kt two n",
                                              two=2, n=HD))
                nc.sync.dma_start(maskT_sb, maskT.ap())
                nc.sync.dma_start(ident_sb, ident.ap())
                nc.vector.memset(ones_k, 1.0 / SA)
                nc.vector.memset(ones4, 1.0)
                return hs_sb

            def gen_chunk0(hs_sb):
                """Block-major chunk 0: five concurrent psum accumulations,
                8-k-tile blocks emitted in the order their DMA pieces land."""
                pss = [wpool.tile([128, CH], F32, tag="work", name=f"ps0_{j}")
                       for j in range(5)]
                projs = [proj_mms(wq_h[m], hs_sb) for m in range(HL)]
                projs.append(proj_mms(wk_sb, hs_sb))
                order = [(0, 0), (1, 0), (0, 1), (1, 1), (2, 0), (2, 1),
                         (0, 2), (1, 2), (2, 2), (3, 0), (3, 1), (3, 2),
                         (0, 3), (1, 3), (2, 3), (3, 3),
                         (4, 0), (4, 1), (4, 2), (4, 3)]
                for j, b in order:
                    for i in range(12 * b, 12 * b + 12):
                        lhsT, rhs = projs[j][i]
                        nc.tensor.matmul(pss[j], lhsT, rhs, start=(i == 0),
                                         stop=(i == 47), perf_mode=DR)
                    yield
                for m in range(HL):
                    rope(pss[m], qt_rot[:, m, 0:CH],
                         cq_sb[:, 0:CH], sq_sb[:, 0:CH])
                    yield
                rope(pss[4], kt_rot[:, 0:CH], ck_sb[:, 0:CH], sk_sb[:, 0:CH])
                yield
                yield from gen_v(0, hs_sb)

            def issue_hs(c):
                t0 = c * CH
                hs_sb = hpool.tile([128, KT, 2, CH], F8, tag="hs",
                                   name="hs_sb")
                for g in range(8):
                    for s in range(2):
                        nc.sync.dma_start(
                            hs_sb[:, g * 4:(g + 1) * 4, s, :],
                            hs_r[:, g * 4:(g + 1) * 4, s, t0:t0 + CH])
                return hs_sb

            prev_groups = []
            hs_cur = issue_hs0()
            for c in range(NCHUNK):
                cg = gen_chunk0(hs_cur) if c == 0 else gen_chunk(c, hs_cur)
                gg = chain([gen_group(*g) for g in prev_groups])
                # units: chunk ~33, groups 24 (qb0) / 40 (qb1)
                glen = (24 if (c - 1) % 2 == 0 else 40) if prev_groups else 0
                acc = 0.0
                qb1_groups = prev_groups and (c - 1) % 2 == 1
                unit = 0
                alive = True
                while alive:
                    alive = next(cg, StopIteration) is not StopIteration
                    unit += 1
                    if unit == PREFETCH_UNIT and c + 1 < NCHUNK:
                        # prefetch next chunk's hs mid-chunk so its first
                        # pieces land before the next projections start
                        hs_cur = issue_hs(c + 1)
                    acc += glen / 33.0
                    while acc >= 1.0:
                        next(gg, None)
                        acc -= 1.0
                for _ in gg:
                    pass
                b, qb = c // 2, c % 2
                prev_groups = [(b, h, qb) for h in range(HL)]
                if c == NCHUNK - 1:
                    # wo reuses an hs slot (same size); DMA on the Act queue
                    # overlaps the final groups
                    wo_sb_box[0] = hpool.tile([128, HL, 2, DIM], F8, tag="hs",
                                              name="wo_sb")
                    for m in range(HL):
                        nc.scalar.dma_start(
                            wo_sb_box[0][:, m, :, :],
                            wo8.ap()[m * 128:(m + 1) * 128, :]
                            .rearrange("p (two n) -> p two n", two=2))

            # tail: last chunk's groups interleaved with O tiles of the
            # earlier chunks, then the rest of the O projection
            o_list = [(tt, ni) for tt in range(TT) for ni in range(8)]
            o_head = 0
            gg = chain([gen_group(*g) for g in prev_groups])
            oacc = 0.0
            for _ in gg:
                oacc += 2.4
                while oacc >= 1.0 and o_head < 96:
                    emit_o_tile(*o_list[o_head])
                    o_head += 1
                    oacc -= 1.0
            if pending[0] is not None:
                epilogue_rest(pending[0])
                pending[0] = None
            while o_head < len(o_list):
                emit_o_tile(*o_list[o_head])
                o_head += 1
    nc.compile()
    return nc


def _get_nc():
    if "nc" not in _CACHE:
        _CACHE["nc"] = _build()
    return _CACHE["nc"]


def _split8(x):
    f8 = ml_dtypes.float8_e4m3
    hi = x.astype(f8)
    lo = (x - hi.astype(np.float32)).astype(f8)
    return hi, lo


def _prep_inputs(inputs) -> list[dict]:
    bf16 = ml_dtypes.bfloat16
    hs = np.asarray(inputs["hidden_states"], dtype=np.float32).reshape(T, DIM)
    hsT = np.ascontiguousarray(hs.T) * SH
    h_hi, h_lo = _split8(hsT)
    hs8 = np.ascontiguousarray(np.stack([h_hi, h_lo], axis=1))  # [DIM, 2, T]

    fc = np.asarray(inputs["freqs_cos"], dtype=np.float32).reshape(T, HD // 2).T
    fs = np.asarray(inputs["freqs_sin"], dtype=np.float32).reshape(T, HD // 2).T
    cos2 = np.concatenate([fc, fc], axis=0)            # [128, T]
    sin2 = np.concatenate([-fs, fs], axis=0)           # signed half-rotation
    qs = SCALE / (SH * SW)
    ks = 1.0 / (SH * SW)
    cos_qv = np.ascontiguousarray(cos2 * qs).astype(bf16)
    sin_qv = np.ascontiguousarray(sin2 * qs).astype(bf16)
    cos_kv = np.ascontiguousarray(cos2 * ks).astype(bf16)
    sin_kv = np.ascontiguousarray(sin2 * ks).astype(bf16)

    maskT = np.ascontiguousarray(
        np.asarray(inputs["attention_mask"], dtype=np.float32)[0, 0, :128, :128].T
    ).astype(ml_dtypes.bfloat16)
    ident = np.eye(128, dtype=np.float32).astype(ml_dtypes.bfloat16)

    perm = np.concatenate([np.arange(0, HD, 2), np.arange(1, HD, 2)])
    Wq = np.asarray(inputs["Wq"], dtype=np.float32)
    Wk = np.asarray(inputs["Wk"], dtype=np.float32)
    Wv = np.asarray(inputs["Wv"], dtype=np.float32)
    Wo = np.asarray(inputs["Wo"], dtype=np.float32)

    def pack_w(w, nheads):
        # w: [DIM, nheads*128] prescaled; -> [nheads*128p, KT*2*HD] with
        # (lo, hi) pairs: arr[m*128+p, ((kt*2)+s)*128+hd]
        hi, lo = _split8(w)
        pair = np.stack([lo.astype(np.float32), hi.astype(np.float32)], axis=1)
        # [DIM, 2, nheads*128] -> [KT, 128p, 2, nheads, 128hd]
        v = pair.reshape(KT, 128, 2, nheads, HD)
        arr = v.transpose(3, 1, 0, 2, 4).reshape(nheads * 128, KT * 2 * HD)
        return np.ascontiguousarray(arr).astype(ml_dtypes.float8_e4m3)

    in_maps = []
    for c in range(N_CORES):
        wq_c = np.concatenate(
            [Wq[:, (c * HL + h) * HD:(c * HL + h + 1) * HD][:, perm]
             for h in range(HL)], axis=1) * SW
        wk_c = Wk[:, c * HD:(c + 1) * HD][:, perm] * SW
        wv_c = Wv[:, c * HD:(c + 1) * HD] * SW
        wo_c = Wo[c * HL * HD:(c + 1) * HL * HD, :] * SW

        o_hi, o_lo = _split8(wo_c)
        wo_pack = np.concatenate([o_lo.astype(np.float32),
                                  o_hi.astype(np.float32)],
                                 axis=1)  # [512, 2*DIM] (lo block, hi block)
        wo_pack = np.ascontiguousarray(wo_pack).astype(ml_dtypes.float8_e4m3)

        in_maps.append({
            "hs8": hs8,
            "wq8": pack_w(wq_c, HL),
            "wk8": pack_w(wk_c, 1),
            "wv8": pack_w(wv_c, 1),
            "wo8": wo_pack,
            "cos_q": cos_qv, "sin_q": sin_qv,
            "cos_k": cos_kv, "sin_k": sin_kv,
            "maskT": maskT,
            "ident": ident,
        })
    return in_maps


def kernel(**inputs) -> np.ndarray:
    nc = _get_nc()
    in_maps = _prep_inputs(inputs)
    res = bass_utils.run_bass_kernel_spmd(nc, in_maps,
                                          core_ids=list(range(N_CORES)))
    acc = np.zeros((T, DIM), dtype=np.float32)
    for c in range(N_CORES):
        acc += np.asarray(res.results[c]["out"], dtype=np.float32)
    return acc.reshape(B, S, DIM)
